# revision 1
# baseline (speedup 1.0000x reference)
"""Trainium2 Bass kernel: 4-layer decoder prefill (S=1024, H=2048, NH=16, HD=128,
FFN=5632, V=32000), tensor-parallel over 8 NeuronCores.

- Megatron TP over 8 cores: wq/wk/wv/w1/w3 sharded on output dim (2 heads /
  704 ffn rows per core), wo/w2 sharded on input dim (partials -> AllReduce),
  out_w sharded over vocab (4000 rows/core); only the last token's logits are
  computed.
- The residual stream lives TRANSPOSED in SBUF (xT: [H on partition-chunks,
  S free]); weights are pre-transposed on the host so every matmul contracts
  over the partition dim with no on-device weight transposes. V is re-
  transposed on the PE so attention*V contracts over key tokens.
- Scores come out directly as [ktok, qtok]; softmax sums are ones-vector
  matmuls on the PE; max-subtraction is skipped (scores are O(+-5)).
- All matmuls run in float32r (full-rate fp32, ~1e-4 rel err).
"""

import os
import sys

sys.path.insert(0, "/opt/trn_rl_repo")

import numpy as np

L = int(os.environ.get("KERNEL_DEV_L", "4"))
SKIP = set(os.environ.get("KERNEL_SKIP", "").split(","))
B, S, H, NH, HD = 1, 1024, 2048, 16, 128
V, P = 32000, 5632
NC = 8
FEAT = H // NC          # 256 q/k/v features per core (2 heads)
PC = P // NC            # 704 ffn rows per core
VC = V // NC            # 4000 vocab rows per core
KH = H // 128           # 16 H-chunks
KP = (PC + 127) // 128  # 6 pc-chunks (last is 64)
EPS = 1e-5
SCALE = float(np.sqrt(HD))
INV_SCALE = 1.0 / SCALE

_STATE = {}


def _build():
    import concourse.bass as bass
    import concourse.bacc as bacc
    from concourse import tile, mybir

    F32 = mybir.dt.float32
    F32R = mybir.dt.float32r
    F16 = mybir.dt.float16
    BF16 = mybir.dt.bfloat16
    AF = mybir.ActivationFunctionType
    ALU = mybir.AluOpType
    ts = bass.ts

    nc = bacc.Bacc("TRN2", target_bir_lowering=False, debug=False, num_devices=NC)

    xT_h = nc.dram_tensor("xT", [H, S], BF16, kind="ExternalInput")
    maskT_h = nc.dram_tensor("maskT", [S, S], F32, kind="ExternalInput")
    mdiag_h = nc.dram_tensor("mdiag", [128, 4 * 512], BF16,
                             kind="ExternalInput")
    C_h = nc.dram_tensor("Cr", [128, S], F32R, kind="ExternalInput")
    S_h = nc.dram_tensor("Sr", [128, S], F32, kind="ExternalInput")
    J_h = nc.dram_tensor("J", [128, 128], F16, kind="ExternalInput")
    id_h = nc.dram_tensor("ident", [128, 128], F16, kind="ExternalInput")
    n1w_h = nc.dram_tensor("n1w", [128, L * KH], F32, kind="ExternalInput")
    n2w_h = nc.dram_tensor("n2w", [128, L * KH], F32, kind="ExternalInput")
    fw_h = nc.dram_tensor("fw", [128, KH], F32, kind="ExternalInput")
    # wq|wk|wv concatenated on the last axis: [L, H, 3*FEAT]
    wqkv_h = nc.dram_tensor("wqkvT", [L, H, 3 * FEAT], F16, kind="ExternalInput")
    woT_h = nc.dram_tensor("woT", [L, FEAT, H], F16, kind="ExternalInput")
    # w1|w3 interleaved by m-group: [w1 0:384 | w3 0:384 | w1 384:704 | w3 384:704]
    w13_h = nc.dram_tensor("w13T", [L, H, 2 * PC], F16, kind="ExternalInput")
    w2T_h = nc.dram_tensor("w2T", [L, PC, H], F16, kind="ExternalInput")
    owT_h = nc.dram_tensor("owT", [H, VC], F16, kind="ExternalInput")
    out_h = nc.dram_tensor("logits", [1, VC], F32, kind="ExternalOutput")

    MW = [128] * (KP - 1) + [PC - 128 * (KP - 1)]   # 128 x5, 64
    MG_OFF = [0, 384]
    MG_WID = [384, PC - 384]

    def memset_r(ap_, w):
        """memset an F32R AP via an F32 staging tile (direct memset is
        illegal on f32r)."""
        stg = p_f32.tile([128, 512], F32, tag="f32t", name="msr")
        nc.vector.memset(stg[:], 0.01)
        for off in range(0, w, 512):
            ww = min(512, w - off)
            nc.vector.tensor_copy(ap_[:, off:off + ww], stg[:, :ww])

    def coll_ar(ins_ap, outs_ap):
        if "coll" in SKIP:
            nc.sync.dma_start(outs_ap, ins_ap)
        else:
            nc.gpsimd.collective_compute(
                "AllReduce", ALU.add, replica_groups=[list(range(NC))],
                ins=[ins_ap.opt()], outs=[outs_ap.opt()])

    def coll_rs_ag(in_ap, mid_ap, out_ap):
        if "coll" in SKIP:
            nc.sync.dma_start(out_ap, in_ap)
        else:
            nc.gpsimd.collective_compute(
                "ReduceScatter", ALU.add, replica_groups=[list(range(NC))],
                ins=[in_ap.opt()], outs=[mid_ap.opt()])
            nc.gpsimd.collective_compute(
                "AllGather", ALU.bypass, replica_groups=[list(range(NC))],
                ins=[mid_ap.opt()], outs=[out_ap.opt()])

    from contextlib import ExitStack

    with tile.TileContext(nc) as tc, ExitStack() as _ctx:
        ec = _ctx.enter_context
        p_resid = ec(tc.tile_pool(name="resid", bufs=1))
        p_const = ec(tc.tile_pool(name="consts", bufs=1))
        p_row = ec(tc.tile_pool(name="row", bufs=1))
        p_big = ec(tc.tile_pool(name="big", bufs=4))
        p_vs = ec(tc.tile_pool(name="vsn", bufs=1))
        p_pt = ec(tc.tile_pool(name="ptile", bufs=3))
        p_f32 = ec(tc.tile_pool(name="f32t", bufs=2))
        p_t512 = ec(tc.tile_pool(name="t512", bufs=3))
        p_ns = ec(tc.tile_pool(name="normsc", bufs=3))
        p_stg = ec(tc.tile_pool(name="stage", bufs=4))
        p_w13 = ec(tc.tile_pool(name="w13", bufs=3))
        p_w2 = ec(tc.tile_pool(name="w2p", bufs=2))
        p_swig = ec(tc.tile_pool(name="swig", bufs=6))
        p_ar = ec(tc.tile_pool(name="ars", bufs=4))
        psum = ec(tc.tile_pool(name="psum", bufs=2, space="PSUM"))
        dram = ec(tc.tile_pool(name="dram", bufs=4, space="DRAM"))

        xT = p_resid.tile([128, KH * S], BF16, tag="xT")
        for hc in range(KH):
            nc.sync.dma_start(xT[:, ts(hc, S)], xT_h.ap()[ts(hc, 128), :])

        C_s = p_const.tile([128, S], F32R, tag="C")
        nc.sync.dma_start(C_s[:], C_h.ap())
        S_s = p_const.tile([128, S], F32, tag="S")
        nc.sync.dma_start(S_s[:], S_h.ap())
        J_r = p_const.tile([128, 128], F16, tag="J")
        nc.sync.dma_start(J_r[:], J_h.ap())
        id_r = p_const.tile([128, 128], F16, tag="id")
        nc.sync.dma_start(id_r[:], id_h.ap())
        n1w = p_const.tile([128, L * KH], F32, tag="n1w")
        nc.sync.dma_start(n1w[:], n1w_h.ap())
        n2w = p_const.tile([128, L * KH], F32, tag="n2w")
        nc.sync.dma_start(n2w[:], n2w_h.ap())
        fw_s = p_const.tile([128, KH], F32, tag="fw")
        nc.sync.dma_start(fw_s[:], fw_h.ap())
        mdiag_s = p_const.tile([128, 4 * 512], BF16, tag="mdiag")
        nc.sync.dma_start(mdiag_s[:], mdiag_h.ap())
        ones_f = p_const.tile([128, 1], F32, tag="o1f")
        nc.vector.memset(ones_f[:], 1.0)
        ones_col = p_const.tile([128, 1], F32R, tag="o1")
        nc.vector.tensor_copy(ones_col[:], ones_f[:])
        ones_rf = p_const.tile([1, 128], F32, tag="orf")
        nc.vector.memset(ones_rf[:], 1.0)
        ones_row = p_const.tile([1, 128], F32R, tag="or")
        nc.vector.tensor_copy(ones_row[:], ones_rf[:])
        eps_t = p_const.tile([1, 1], F32, tag="eps")
        nc.vector.memset(eps_t[:], EPS)
        eps_p = p_const.tile([128, 1], F32, tag="epsp")
        nc.vector.memset(eps_p[:], EPS)
        ones_mf = p_const.tile([128, 128], F32, tag="omf")
        nc.vector.memset(ones_mf[:], 1.0)
        ones_mat = p_const.tile([128, 128], F32R, tag="om")
        nc.vector.tensor_copy(ones_mat[:], ones_mf[:])
        ones_mh = p_const.tile([128, 128], F16, tag="omh")
        nc.vector.tensor_copy(ones_mh[:], ones_mf[:])
        ones_ch = p_const.tile([128, 1], F16, tag="o1h")
        nc.vector.tensor_copy(ones_ch[:], ones_f[:])

        def norm_half(w_tile, l_, tk):
            """1/rms for tokens [tk*512, tk*512+512), bcast -> bc_s [128,512]."""
            ssum = psum.tile([1, 512], F32, tag="acc", bufs=6, name="ssum")
            for hc in range(KH):
                sq = p_pt.tile([128, 512], F32R, tag="pt", name="sq")
                sl = slice(hc * S + tk * 512, hc * S + tk * 512 + 512)
                nc.vector.tensor_mul(sq[:], xT[:, sl], xT[:, sl])
                nc.tensor.matmul(ssum[:], ones_col[:], sq[:],
                                 start=(hc == 0), stop=(hc == KH - 1))
            rms = p_row.tile([1, 512], F32, tag="rms")
            nc.scalar.activation(rms[:], ssum[:], AF.Sqrt,
                                 bias=eps_t[:], scale=1.0 / H)
            inv = p_row.tile([1, 512], F32R, tag="inv")
            with nc.allow_low_precision(reason="f32r rounding of 1/rms"):
                nc.vector.reciprocal(inv[:], rms[:])
            bc_ps = psum.tile([128, 512], F32, tag="ps512", name="bcps")
            nc.tensor.matmul(bc_ps[:], ones_row[:], inv[:], start=True, stop=True)
            bc_s = p_f32.tile([128, 512], F32, tag="f32t", name="bcs")
            nc.scalar.activation(bc_s[:], bc_ps[:], AF.Copy)
            return bc_s

        def qkv_half(l_, tk, q_s, k_s, vT_s):
            """QKV for token half tk of layer l_ (writes [:, mt*S + tk*512])."""
            if "qkv" in SKIP:
                for mt in range(2):
                    off = mt * S + tk * 512
                    for t_s in (q_s, k_s, vT_s):
                        memset_r(t_s[:, off:off + 512], 512)
                return
            bc = norm_half(n1w, l_, tk)
            qp = [psum.tile([128, 512], F32, tag="acc", bufs=6, name=f"qp{i}")
                  for i in range(2)]
            kp = [psum.tile([128, 512], F32, tag="acc", bufs=6, name=f"kp{i}")
                  for i in range(2)]
            vp = [psum.tile([128, 512], F32, tag="acc", bufs=6, name=f"vp{i}")
                  for i in range(2)]
            for hc in range(KH):
                xn = p_ns.tile([128, 512], F16, tag="ns", name="xn")
                nc.vector.scalar_tensor_tensor(
                    xn[:], xT[:, hc * S + tk * 512: hc * S + tk * 512 + 512],
                    n1w[:, l_ * KH + hc: l_ * KH + hc + 1],
                    bc[:], op0=ALU.mult, op1=ALU.mult)
                wt = p_w13.tile([128, 3 * FEAT], F16, tag="w13", name="wt")
                nc.sync.dma_start(wt[:], wqkv_h.ap()[l_, ts(hc, 128), :])
                st, sp = (hc == 0), (hc == KH - 1)
                for mt in range(2):
                    nc.tensor.matmul(qp[mt][:], wt[:, ts(mt, 128)], xn[:],
                                     start=st, stop=sp)
                    nc.tensor.matmul(kp[mt][:], wt[:, 256 + mt * 128: 384 + mt * 128],
                                     xn[:], start=st, stop=sp)
                    nc.tensor.matmul(vp[mt][:], wt[:, 512 + mt * 128: 640 + mt * 128],
                                     xn[:], start=st, stop=sp)
            for mt in range(2):
                off = mt * S + tk * 512
                nc.vector.tensor_copy(q_s[:, off:off + 512], qp[mt][:])
                nc.vector.tensor_copy(k_s[:, off:off + 512], kp[mt][:])
                nc.vector.tensor_copy(vT_s[:, off:off + 512], vp[mt][:])

        # ---- layer 0 QKV prologue ----
        cur_q = p_big.tile([128, 2 * S], F16, tag="big", name="q0")
        cur_k = p_big.tile([128, 2 * S], F16, tag="big", name="k0")
        cur_vT = p_big.tile([128, 2 * S], F16, tag="big", name="vT0")
        attn_s = p_big.tile([128, 2 * S], F16, tag="big", name="attn")
        for tk in range(2):
            qkv_half(0, tk, cur_q, cur_k, cur_vT)

        for l in range(L):
            last = (l == L - 1)
            q_s, k_s, vT_s = cur_q, cur_k, cur_vT

            # RoPE in place on q_s, k_s:  out = C*x + S'*(J@x)
            for t_s in ((q_s, k_s) if "rope" not in SKIP else ()):
                for mt in range(2):
                    for n in range(2):
                        sl = slice(mt * S + n * 512, mt * S + n * 512 + 512)
                        csl = slice(n * 512, n * 512 + 512)
                        j_ps = psum.tile([128, 512], F32, tag="ps512", name="jps")
                        nc.tensor.matmul(j_ps[:], J_r[:], t_s[:, sl],
                                         start=True, stop=True)
                        tmp = p_t512.tile([128, 512], F16, tag="t512r",
                                          name="rtmp")
                        nc.vector.tensor_mul(tmp[:], C_s[:, csl], t_s[:, sl])
                        nc.vector.tensor_mul(t_s[:, sl], j_ps[:], S_s[:, csl])
                        nc.vector.tensor_add(t_s[:, sl], t_s[:, sl], tmp[:])

            # V -> natural layout [tok, feat] via PE transpose
            v_s = p_vs.tile([128, 8 * FEAT], F16, tag="v", name="vs")
            if "vtr" in SKIP:
                memset_r(v_s[:], 8 * FEAT)
            for mt in range(2 if "vtr" not in SKIP else 0):
                for tb in range(8):
                    tp = psum.tile([128, 128], F16, tag="ps512", name="tp")
                    nc.tensor.transpose(
                        tp[:], vT_s[:, mt * S + tb * 128: mt * S + tb * 128 + 128],
                        id_r[:])
                    nc.vector.tensor_copy(
                        v_s[:, tb * FEAT + mt * 128: tb * FEAT + mt * 128 + 128],
                        tp[:])

            if last:
                # only the last token's query matters (2-wide for ISA).
                # kc<7 blocks are fully causal-visible; only kc=7 needs mask.
                mlast = p_t512.tile([128, 2], F32, tag="mk1", name="mlast")
                nc.sync.dma_start(mlast[:], maskT_h.ap()[ts(7, 128), S - 2: S])
                for h in range(2):
                    at1 = psum.tile([128, 2], F32, tag="acc", bufs=6, name="at1")
                    rs1 = psum.tile([128, 2], F32, tag="acc", bufs=6, name="rs1")
                    for kc in range(8):
                        sc1 = psum.tile([128, 2], F32, tag="ps512", name="sc1")
                        nc.tensor.matmul(
                            sc1[:],
                            k_s[:, h * S + kc * 128: h * S + kc * 128 + 128],
                            q_s[:, h * S + S - 2: h * S + S],
                            start=True, stop=True)
                        pt1 = p_t512.tile([128, 2], F16, tag="mk1", name="pt1")
                        if kc == 7:
                            ex1 = p_t512.tile([128, 2], F32, tag="mk1",
                                              name="ex1")
                            nc.vector.scalar_tensor_tensor(
                                ex1[:], sc1[:], INV_SCALE, mlast[:],
                                op0=ALU.mult, op1=ALU.add)
                            nc.scalar.activation(pt1[:], ex1[:], AF.Exp)
                        else:
                            nc.scalar.activation(pt1[:], sc1[:], AF.Exp,
                                                 scale=INV_SCALE)
                        st, sp = (kc == 0), (kc == 7)
                        nc.tensor.matmul(
                            at1[:],
                            v_s[:, kc * FEAT + h * 128: kc * FEAT + h * 128 + 128],
                            pt1[:], start=st, stop=sp)
                        nc.tensor.matmul(rs1[:], ones_mh[:], pt1[:],
                                         start=st, stop=sp)
                    inva = p_t512.tile([128, 2], F32, tag="mk1", name="inva")
                    nc.vector.reciprocal(inva[:], rs1[:])
                    nc.vector.tensor_mul(
                        attn_s[:, h * S + S - 2: h * S + S], at1[:], inva[:])

                # wo -> [H,2] AllReduce -> residual add (last token)
                ar_in = dram.tile([128, 2 * KH], F16, tag="arinL", name="arinL")
                ar_out = dram.tile([128, 2 * KH], F16, tag="aroutL",
                                   addr_space="Shared", name="aroutL")
                arwL = p_ar.tile([128, 2 * KH], F16, tag="arL", name="arwL")
                woL = [p_stg.tile([128, H], F16, tag="stg",
                                  name=f"woL{i}") for i in range(2)]
                for fc in range(2):
                    nc.sync.dma_start(woL[fc][:], woT_h.ap()[l, ts(fc, 128), :])
                for hc in range(KH):
                    poL = psum.tile([128, 2], F32, tag="ps512", name="poL")
                    for fc in range(2):
                        nc.tensor.matmul(
                            poL[:], woL[fc][:, ts(hc, 128)],
                            attn_s[:, fc * S + S - 2: fc * S + S],
                            start=(fc == 0), stop=(fc == 1))
                    nc.scalar.activation(arwL[:, 2 * hc: 2 * hc + 2], poL[:],
                                         AF.Copy)
                nc.sync.dma_start(ar_in[:], arwL[:])
                coll_ar(ar_in[:], ar_out[:])
                arrL = p_ar.tile([128, 2 * KH], F16, tag="arL", name="arrL")
                nc.sync.dma_start(arrL[:], ar_out[:])
                for hc in range(KH):
                    nc.vector.tensor_add(
                        xT[:, hc * S + S - 2: hc * S + S],
                        xT[:, hc * S + S - 2: hc * S + S],
                        arrL[:, 2 * hc: 2 * hc + 2])

                # norm2 + FFN on the last 2 tokens
                sqL = p_row.tile([128, 2 * KH], F32R, tag="sql2")
                for hc in range(KH):
                    col = hc * S + S - 2
                    nc.vector.tensor_mul(sqL[:, 2 * hc:2 * hc + 2],
                                         xT[:, col:col + 2], xT[:, col:col + 2])
                ssL = psum.tile([128, 2 * KH], F32, tag="ps512", name="ssL")
                nc.tensor.matmul(ssL[:], ones_mat[:], sqL[:],
                                 start=True, stop=True)
                ssr = p_row.tile([128, 2], F32, tag="ssr")
                nc.vector.reduce_sum(
                    ssr[:], ssL[:].rearrange("p (c two) -> p two c", two=2),
                    axis=mybir.AxisListType.X)
                rmsL = p_row.tile([128, 2], F32, tag="rmsL")
                nc.scalar.activation(rmsL[:], ssr[:], AF.Sqrt,
                                     bias=eps_p[:], scale=1.0 / H)
                invL = p_row.tile([128, 2], F32, tag="invLc")
                nc.vector.reciprocal(invL[:], rmsL[:])
                hnL = p_row.tile([128, 2 * KH], F16, tag="hnL")
                tnL = p_row.tile([128, 2], F32, tag="tnL")
                for hc in range(KH):
                    col = hc * S + S - 2
                    nc.vector.tensor_scalar_mul(
                        tnL[:], xT[:, col:col + 2],
                        n2w[:, l * KH + hc: l * KH + hc + 1])
                    nc.vector.tensor_mul(hnL[:, 2 * hc:2 * hc + 2],
                                         tnL[:], invL[:])
                swigL = p_row.tile([128, 2 * KP], F16, tag="swL")
                for mg in range(2):
                    mts = [0, 1, 2] if mg == 0 else [3, 4, 5]
                    w_off, w_wid = MG_OFF[mg], MG_WID[mg]
                    gL = {mt: psum.tile([128, 2], F32, tag="acc", bufs=6,
                                        name=f"gL{mt}") for mt in mts}
                    uL = {mt: psum.tile([128, 2], F32, tag="acc", bufs=6,
                                        name=f"uL{mt}") for mt in mts}
                    for hc in range(KH):
                        wt13 = p_w13.tile([128, 2 * 384], F16, tag="w13",
                                          name="wt13L")
                        nc.sync.dma_start(
                            wt13[:, :2 * w_wid],
                            w13_h.ap()[l, ts(hc, 128),
                                       2 * w_off: 2 * w_off + 2 * w_wid])
                        st, sp = (hc == 0), (hc == KH - 1)
                        for i, mt in enumerate(mts):
                            w = min(128, w_wid - i * 128)
                            nc.tensor.matmul(
                                gL[mt][:w, :], wt13[:, i * 128: i * 128 + w],
                                hnL[:, 2 * hc:2 * hc + 2], start=st, stop=sp)
                            nc.tensor.matmul(
                                uL[mt][:w, :],
                                wt13[:, w_wid + i * 128: w_wid + i * 128 + w],
                                hnL[:, 2 * hc:2 * hc + 2], start=st, stop=sp)
                    for mt in mts:
                        kw = MW[mt]
                        gsL = p_row.tile([128, 2], F16, tag="gsL")
                        nc.scalar.activation(gsL[:kw, :], gL[mt][:kw, :], AF.Silu)
                        nc.vector.tensor_mul(swigL[:kw, 2 * mt:2 * mt + 2],
                                             uL[mt][:kw, :], gsL[:kw, :])
                ar2_in = dram.tile([128, 2 * KH], F16, tag="arinL",
                                   name="ar2inL")
                ar2_out = dram.tile([128, 2 * KH], F16, tag="aroutL",
                                    addr_space="Shared", name="ar2outL")
                arw2L = p_ar.tile([128, 2 * KH], F16, tag="arL", name="arw2L")
                for hcb in range(4):
                    p2L = [psum.tile([128, 2], F32, tag="acc", bufs=6,
                                     name=f"p2L{i}") for i in range(4)]
                    for kc in range(KP):
                        kw = MW[kc]
                        w2_t = p_w2.tile([128, 512], F16, tag="w2",
                                         name="w2tL")
                        nc.sync.dma_start(
                            w2_t[:kw, :],
                            w2T_h.ap()[l, kc * 128: kc * 128 + kw,
                                       hcb * 512: hcb * 512 + 512])
                        for hh in range(4):
                            nc.tensor.matmul(
                                p2L[hh][:], w2_t[:kw, ts(hh, 128)],
                                swigL[:kw, 2 * kc:2 * kc + 2],
                                start=(kc == 0), stop=(kc == KP - 1))
                    for hh in range(4):
                        hc = hcb * 4 + hh
                        nc.scalar.activation(arw2L[:, 2 * hc: 2 * hc + 2],
                                             p2L[hh][:], AF.Copy)
                nc.sync.dma_start(ar2_in[:], arw2L[:])
                coll_ar(ar2_in[:], ar2_out[:])
                arr2L = p_ar.tile([128, 2 * KH], F16, tag="arL", name="arr2L")
                nc.sync.dma_start(arr2L[:], ar2_out[:])
                for hc in range(KH):
                    nc.vector.tensor_add(
                        xT[:, hc * S + S - 2: hc * S + S],
                        xT[:, hc * S + S - 2: hc * S + S],
                        arr2L[:, 2 * hc: 2 * hc + 2])
                continue

            # ---- non-last layer: attention for both halves, then the
            # token-half-pipelined tail (wo->AR1->norm2->FFN->AR2->next QKV)
            if "attn" in SKIP:
                memset_r(attn_s[:], 2 * S)
            for tk in range(2 if "attn" not in SKIP else 0):
                # causal: query half tk only attends key blocks kc*128 <
                # (tk+1)*512. Fully-visible blocks exp straight from PSUM;
                # diagonal blocks add the preloaded [128,512] mask pattern d.
                if tk == 0:
                    blocks = [(kc, kc) for kc in range(4)]
                else:
                    blocks = ([(kc, None) for kc in range(4)]
                              + [(kc, kc - 4) for kc in range(4, 8)])
                nb = len(blocks)
                at_ps, rs_ps = {}, {}
                for h in range(2):
                    at_ps[h] = psum.tile([128, 512], F32, tag="acc", bufs=6,
                                         name=f"atp{h}")
                    rs_ps[h] = psum.tile([1, 512], F32, tag="acc", bufs=6,
                                         name=f"rsp{h}")
                for bi, (kc, d) in enumerate(blocks):
                    sc = {}
                    for h in range(2):
                        sc[h] = psum.tile([128, 512], F32, tag="ps512",
                                          name=f"scp{h}")
                        nc.tensor.matmul(
                            sc[h][:],
                            k_s[:, h * S + kc * 128: h * S + kc * 128 + 128],
                            q_s[:, h * S + tk * 512: h * S + tk * 512 + 512],
                            start=True, stop=True)
                    pts = {}
                    for h in range(2):
                        pt = p_pt.tile([128, 512], F16, tag="pt", name="ptl")
                        if d is None:
                            nc.scalar.activation(pt[:], sc[h][:], AF.Exp,
                                                 scale=INV_SCALE)
                        else:
                            ex = p_t512.tile([128, 512], F32, tag="t512f",
                                             name="ex")
                            nc.vector.scalar_tensor_tensor(
                                ex[:], sc[h][:], INV_SCALE,
                                mdiag_s[:, ts(d, 512)],
                                op0=ALU.mult, op1=ALU.add)
                            nc.scalar.activation(pt[:], ex[:], AF.Exp)
                        pts[h] = pt
                    st, sp = (bi == 0), (bi == nb - 1)
                    for h in range(2):
                        nc.tensor.matmul(
                            at_ps[h][:],
                            v_s[:, kc * FEAT + h * 128: kc * FEAT + h * 128 + 128],
                            pts[h][:], start=st, stop=sp)
                        nc.tensor.matmul(rs_ps[h][:], ones_ch[:], pts[h][:],
                                         start=st, stop=sp)
                for h in range(2):
                    inv = p_row.tile([1, 512], F32R, tag="inv", name="ainv")
                    with nc.allow_low_precision(reason="f32r 1/sum"):
                        nc.vector.reciprocal(inv[:], rs_ps[h][:])
                    ib_ps = psum.tile([128, 512], F32, tag="ps512", name="ibp")
                    nc.tensor.matmul(ib_ps[:], ones_row[:], inv[:],
                                     start=True, stop=True)
                    ib_s = p_f32.tile([128, 512], F32, tag="f32t", name="ibs")
                    nc.scalar.activation(ib_s[:], ib_ps[:], AF.Copy)
                    nc.vector.tensor_mul(
                        attn_s[:, h * S + tk * 512: h * S + tk * 512 + 512],
                        at_ps[h][:], ib_s[:])

            ar1_bufs = []
            ar2_bufs = []
            wo_t = None
            for tk in range(2):
                # wo projection for this token half; partials staged as one
                # contiguous [128, KH*512] f16 block (layout-agnostic for the
                # elementwise AR) so each AR needs 1 write + 1 read DMA.
                ar_in = dram.tile([128, KH * 512], F16, tag="arin", name="arin")
                ar_out = dram.tile([128, KH * 512], F16, tag="arout",
                                   addr_space="Shared", name="arout")
                ar1_bufs.append((ar_in, ar_out))
                arw = [p_stg.tile([128, 8 * 512], F16, tag="stg",
                                  name=f"arw{i}") for i in range(2)]
                if "wo" in SKIP:
                    nc.vector.memset(arw[0][:], 0.01)
                    nc.vector.memset(arw[1][:], 0.01)
                else:
                    if wo_t is None:
                        wo_t = [p_stg.tile([128, H], F16, tag="stg",
                                           name=f"wof{i}") for i in range(2)]
                        for fc in range(2):
                            nc.sync.dma_start(wo_t[fc][:],
                                              woT_h.ap()[l, ts(fc, 128), :])
                    for hc in range(KH):
                        po = psum.tile([128, 512], F32, tag="ps512", name="po")
                        for fc in range(2):
                            nc.tensor.matmul(
                                po[:], wo_t[fc][:, ts(hc, 128)],
                                attn_s[:, fc * S + tk * 512: fc * S + tk * 512 + 512],
                                start=(fc == 0), stop=(fc == 1))
                        nc.scalar.activation(arw[hc // 8][:, ts(hc % 8, 512)],
                                             po[:], AF.Copy)
                for i in range(2):
                    nc.sync.dma_start(ar_in[:, ts(i, 8 * 512)], arw[i][:])
                ar_mid = dram.tile([16, KH * 512], F16, tag="armid",
                                   name="armid")
                coll_rs_ag(ar_in[:], ar_mid[:], ar_out[:])

            for tk in range(2):
                ar_in, ar_out = ar1_bufs[tk]
                arr = [p_stg.tile([128, 8 * 512], F16, tag="stg",
                                  name=f"arr{i}") for i in range(2)]
                for i in range(2):
                    nc.sync.dma_start(arr[i][:], ar_out[:, ts(i, 8 * 512)])
                for hc in range(KH):
                    nc.vector.tensor_add(
                        xT[:, hc * S + tk * 512: hc * S + tk * 512 + 512],
                        xT[:, hc * S + tk * 512: hc * S + tk * 512 + 512],
                        arr[hc // 8][:, ts(hc % 8, 512)])

                # norm2 + FFN for this half
                bc2 = norm_half(n2w, l, tk)
                swig = [p_swig.tile([128, 512], F16, tag="sw",
                                    name=f"swig{i}") for i in range(KP)]
                if "f13" in SKIP:
                    for i in range(KP):
                        memset_r(swig[i][:], 512)
                for mg in range(2 if "f13" not in SKIP else 0):
                    mts = [0, 1, 2] if mg == 0 else [3, 4, 5]
                    w_off, w_wid = MG_OFF[mg], MG_WID[mg]
                    gp = {mt: psum.tile([128, 512], F32, tag="acc", bufs=6,
                                        name=f"gp{mt}") for mt in mts}
                    up = {mt: psum.tile([128, 512], F32, tag="acc", bufs=6,
                                        name=f"up{mt}") for mt in mts}
                    for hc in range(KH):
                        hn = p_ns.tile([128, 512], F16, tag="ns", name="hn")
                        nc.vector.scalar_tensor_tensor(
                            hn[:],
                            xT[:, hc * S + tk * 512: hc * S + tk * 512 + 512],
                            n2w[:, l * KH + hc: l * KH + hc + 1],
                            bc2[:], op0=ALU.mult, op1=ALU.mult)
                        wt13 = p_w13.tile([128, 2 * 384], F16, tag="w13",
                                          name="wt13")
                        nc.sync.dma_start(
                            wt13[:, :2 * w_wid],
                            w13_h.ap()[l, ts(hc, 128),
                                       2 * w_off: 2 * w_off + 2 * w_wid])
                        st, sp = (hc == 0), (hc == KH - 1)
                        for i, mt in enumerate(mts):
                            w = min(128, w_wid - i * 128)
                            nc.tensor.matmul(
                                gp[mt][:w, :], wt13[:, i * 128: i * 128 + w],
                                hn[:], start=st, stop=sp)
                            nc.tensor.matmul(
                                up[mt][:w, :],
                                wt13[:, w_wid + i * 128: w_wid + i * 128 + w],
                                hn[:], start=st, stop=sp)
                    for i, mt in enumerate(mts):
                        w = MW[mt]
                        gs = p_t512.tile([128, 512], F16, tag="t512f", name="gs")
                        nc.scalar.activation(gs[:w, :], gp[mt][:w, :], AF.Silu)
                        nc.vector.tensor_mul(
                            swig[mt][:w, :], up[mt][:w, :], gs[:w, :])

                # down projection for this half
                ar2_in = dram.tile([128, KH * 512], F16, tag="arin",
                                   name="ar2in")
                ar2_out = dram.tile([128, KH * 512], F16, tag="arout",
                                    addr_space="Shared", name="ar2out")
                ar2_bufs.append((ar2_in, ar2_out))
                arw2 = [p_stg.tile([128, 8 * 512], F16, tag="stg",
                                   name=f"arw2{i}") for i in range(2)]
                if "f2" in SKIP:
                    nc.vector.memset(arw2[0][:], 0.01)
                    nc.vector.memset(arw2[1][:], 0.01)
                for hcb in range(4 if "f2" not in SKIP else 0):
                    p2 = [psum.tile([128, 512], F32, tag="acc", bufs=6,
                                    name=f"p2p{i}") for i in range(4)]
                    for kc in range(KP):
                        kw = MW[kc]
                        w2_t = p_w2.tile([128, 512], F16, tag="w2", name="w2t")
                        nc.sync.dma_start(
                            w2_t[:kw, :],
                            w2T_h.ap()[l, kc * 128: kc * 128 + kw,
                                       hcb * 512: hcb * 512 + 512])
                        for hh in range(4):
                            nc.tensor.matmul(
                                p2[hh][:], w2_t[:kw, ts(hh, 128)],
                                swig[kc][:kw, :],
                                start=(kc == 0), stop=(kc == KP - 1))
                    for hh in range(4):
                        hc = hcb * 4 + hh
                        nc.scalar.activation(arw2[hc // 8][:, ts(hc % 8, 512)],
                                             p2[hh][:], AF.Copy)
                for i in range(2):
                    nc.sync.dma_start(ar2_in[:, ts(i, 8 * 512)], arw2[i][:])
                ar2_mid = dram.tile([16, KH * 512], F16, tag="armid",
                                    name="ar2mid")
                coll_rs_ag(ar2_in[:], ar2_mid[:], ar2_out[:])

            for tk in range(2):
                ar2_in, ar2_out = ar2_bufs[tk]
                arr2 = [p_stg.tile([128, 8 * 512], F16, tag="stg",
                                   name=f"arr2{i}") for i in range(2)]
                for i in range(2):
                    nc.sync.dma_start(arr2[i][:], ar2_out[:, ts(i, 8 * 512)])
                for hc in range(KH):
                    nc.vector.tensor_add(
                        xT[:, hc * S + tk * 512: hc * S + tk * 512 + 512],
                        xT[:, hc * S + tk * 512: hc * S + tk * 512 + 512],
                        arr2[hc // 8][:, ts(hc % 8, 512)])

                # next layer's QKV for this half (overlaps the other AR);
                # writes the same q/k/vT buffers in place — all reads of the
                # current layer's q/k/v finished at attention time.
                if l + 1 < L:
                    qkv_half(l + 1, tk, cur_q, cur_k, cur_vT)

        # ======== final norm (last token only) + logits ========
        sq_l = p_row.tile([128, KH], F32R, tag="sql")
        for hc in range(KH):
            col = hc * S + S - 1
            nc.vector.tensor_mul(sq_l[:, hc:hc + 1], xT[:, col:col + 1],
                                 xT[:, col:col + 1])
        sl_ps = psum.tile([1, KH], F32, tag="ps512", name="slps")
        nc.tensor.matmul(sl_ps[:], ones_col[:], sq_l[:], start=True, stop=True)
        ssc = p_row.tile([1, 1], F32, tag="ssc")
        nc.vector.reduce_sum(ssc[:], sl_ps[:], axis=mybir.AxisListType.X)
        rms_l = p_row.tile([1, 1], F32, tag="rmsl")
        nc.scalar.activation(rms_l[:], ssc[:], AF.Sqrt, bias=eps_t[:],
                             scale=1.0 / H)
        inv_l = p_row.tile([1, 1], F32, tag="invl")
        nc.vector.reciprocal(inv_l[:], rms_l[:])
        xnl = p_row.tile([128, KH], F16, tag="xnl")
        for hc in range(KH):
            col = hc * S + S - 1
            nc.vector.tensor_mul(xnl[:, hc:hc + 1], xT[:, col:col + 1],
                                 fw_s[:, hc:hc + 1])
        # vocab in 2 passes of 4x500 columns; out_w streamed in [128, 2000]
        # tiles (1MB DMAs) with 4 live [1,500] psum accumulators per pass.
        for vp in range(2):
            voff = vp * 2000
            lg_ps = [psum.tile([1, 500], F32, tag="acc", bufs=6, name=f"lgps{n}")
                     for n in range(4)]
            for hc in range(KH):
                ow_t = p_stg.tile([128, 2000], F16, tag="stg", name="owt")
                nc.sync.dma_start(
                    ow_t[:], owT_h.ap()[ts(hc, 128), voff: voff + 2000])
                for n in range(4):
                    nc.tensor.matmul(lg_ps[n][:], xnl[:, hc: hc + 1],
                                     ow_t[:, ts(n, 500)],
                                     start=(hc == 0), stop=(hc == KH - 1))
            for n in range(4):
                lg = p_row.tile([1, 500], F32, tag="lg")
                nc.scalar.activation(lg[:], lg_ps[n][:], AF.Copy,
                                     scale=inv_l[:])
                nc.sync.dma_start(
                    out_h.ap()[:, voff + n * 500: voff + n * 500 + 500], lg[:])

    nc.compile()
    return nc


def _shard(inputs):
    x = np.asarray(inputs["x"], np.float32)
    mask = np.asarray(inputs["attn_mask"], np.float32)
    cos = np.asarray(inputs["cos"], np.float32).reshape(S, HD // 2)
    sin = np.asarray(inputs["sin"], np.float32).reshape(S, HD // 2)
    n1 = np.asarray(inputs["norm1_w"], np.float32)[:L]
    n2 = np.asarray(inputs["norm2_w"], np.float32)[:L]
    fw = np.asarray(inputs["final_norm_w"], np.float32)
    wq = np.asarray(inputs["wq"], np.float32)[:L]
    wk = np.asarray(inputs["wk"], np.float32)[:L]
    wv = np.asarray(inputs["wv"], np.float32)[:L]
    wo = np.asarray(inputs["wo"], np.float32)[:L]
    w1 = np.asarray(inputs["w1"], np.float32)[:L]
    w3 = np.asarray(inputs["w3"], np.float32)[:L]
    w2 = np.asarray(inputs["w2"], np.float32)[:L]
    ow = np.asarray(inputs["out_w"], np.float32)

    import ml_dtypes

    xT = np.ascontiguousarray(x[0].T).astype(ml_dtypes.bfloat16)
    maskT = np.ascontiguousarray(mask[0].T)
    # diagonal-block causal masks: pattern d covers key block kc with
    # kc % 4 == d against a 512-query half; 0 where visible else -1e9
    kl = np.arange(128)[:, None]
    qq = np.arange(512)[None, :]
    mdiag = np.concatenate(
        [np.where(kl + 128 * d <= qq, 0.0, -1e9) for d in range(4)],
        axis=1).astype(ml_dtypes.bfloat16)
    C = np.empty((128, S), np.float32)
    C[0::2] = cos.T
    C[1::2] = cos.T
    Sm = np.empty((128, S), np.float32)
    Sm[0::2] = -sin.T
    Sm[1::2] = sin.T
    J = np.zeros((128, 128), np.float16)
    idx = np.arange(0, 128, 2)
    J[idx, idx + 1] = 1.0
    J[idx + 1, idx] = 1.0
    ident = np.eye(128, dtype=np.float16)
    n1w = np.ascontiguousarray(
        n1.reshape(L, KH, 128).transpose(2, 0, 1).reshape(128, L * KH))
    n2w = np.ascontiguousarray(
        n2.reshape(L, KH, 128).transpose(2, 0, 1).reshape(128, L * KH))
    fwh = np.ascontiguousarray(fw.reshape(KH, 128).T)

    common = dict(xT=xT, maskT=maskT, mdiag=mdiag, Cr=C, Sr=Sm, J=J,
                  ident=ident, n1w=n1w, n2w=n2w, fw=fwh)
    in_maps = []
    for c in range(NC):
        fs = slice(c * FEAT, (c + 1) * FEAT)
        ps = slice(c * PC, (c + 1) * PC)
        vs = slice(c * VC, (c + 1) * VC)
        m = dict(common)
        wqT = wq[:, fs, :].transpose(0, 2, 1)
        wkT = wk[:, fs, :].transpose(0, 2, 1)
        wvT = wv[:, fs, :].transpose(0, 2, 1)
        m["wqkvT"] = np.ascontiguousarray(
            np.concatenate([wqT, wkT, wvT], axis=2)).astype(np.float16)
        m["woT"] = np.ascontiguousarray(
            wo[:, :, fs].transpose(0, 2, 1)).astype(np.float16)
        w1T = w1[:, ps, :].transpose(0, 2, 1)
        w3T = w3[:, ps, :].transpose(0, 2, 1)
        m["w13T"] = np.ascontiguousarray(np.concatenate(
            [w1T[:, :, 0:384], w3T[:, :, 0:384],
             w1T[:, :, 384:], w3T[:, :, 384:]], axis=2)).astype(np.float16)
        m["w2T"] = np.ascontiguousarray(
            w2[:, :, ps].transpose(0, 2, 1)).astype(np.float16)
        m["owT"] = np.ascontiguousarray(ow[vs, :].T).astype(np.float16)
        in_maps.append(m)
    return in_maps


def kernel(**inputs) -> np.ndarray:
    from concourse import bass_utils

    if "nc" not in _STATE:
        _STATE["nc"] = _build()
    in_maps = _shard(inputs)
    res = bass_utils.run_bass_kernel_spmd(
        _STATE["nc"], in_maps, core_ids=list(range(NC)))
    out = np.concatenate(
        [res.results[c]["logits"] for c in range(NC)], axis=1)
    return out.astype(np.float32)



# revision 16
# speedup vs baseline: 1.0639x; 1.0639x over previous
"""Trainium2 Bass kernel: 4-layer decoder prefill (S=1024, H=2048, NH=16, HD=128,
FFN=5632, V=32000), tensor-parallel over 8 NeuronCores.

- Megatron TP over 8 cores: wq/wk/wv/w1/w3 sharded on output dim (2 heads /
  704 ffn rows per core), wo/w2 sharded on input dim (partials -> RS+AG),
  out_w sharded over vocab (4000 rows/core); only the last token's logits are
  computed.
- The residual stream lives TRANSPOSED in SBUF (xT: [H on partition-chunks,
  S free]); weights are pre-transposed on the host so every matmul contracts
  over the partition dim with no on-device weight transposes.
- RMS normalization is DEFERRED past the QKV projection: the norm weights are
  folded into wq/wk/wv/w1/w3/out_w on the host, and the per-token 1/rms is
  folded into the RoPE cos/sin tables (rope is linear) and into the V
  PE-transpose evacuation (tensor_scalar mul), so the QKV matmuls read the
  raw residual xT directly with no elementwise prologue.
- 1/rms comes from a single ACT Rsqrt; softmax 1/sum stays on DVE f32r.
- All activations are f16 (2x DVE modes); matmuls accumulate in f32 PSUM.
"""

import os
import sys

sys.path.insert(0, "/opt/trn_rl_repo")

import numpy as np

L = int(os.environ.get("KERNEL_DEV_L", "4"))
B, S, H, NH, HD = 1, 1024, 2048, 16, 128
V, P = 32000, 5632
NC = 8
FEAT = H // NC          # 256 q/k/v features per core (2 heads)
PC = P // NC            # 704 ffn rows per core
VC = V // NC            # 4000 vocab rows per core
KH = H // 128           # 16 H-chunks
KP = (PC + 127) // 128  # 6 pc-chunks (last is 64)
EPS = 1e-5
SCALE = float(np.sqrt(HD))
INV_SCALE = 1.0 / SCALE
NEG = -30000.0          # f16-safe mask value

_STATE = {}


def _build():
    import concourse.bass as bass
    import concourse.bacc as bacc
    from concourse import tile, mybir

    F32 = mybir.dt.float32
    F32R = mybir.dt.float32r
    F16 = mybir.dt.float16
    AF = mybir.ActivationFunctionType
    ALU = mybir.AluOpType
    ts = bass.ts

    nc = bacc.Bacc("TRN2", target_bir_lowering=False, debug=False, num_devices=NC)

    xT_h = nc.dram_tensor("xT", [H, S], F16, kind="ExternalInput")
    mlast_h = nc.dram_tensor("mlast", [128, 2], F32, kind="ExternalInput")
    mdiag_h = nc.dram_tensor("mdiag", [128, 4 * 512], F16, kind="ExternalInput")
    C_h = nc.dram_tensor("Cr", [128, S], F16, kind="ExternalInput")
    S_h = nc.dram_tensor("Sr", [128, S], F16, kind="ExternalInput")
    J_h = nc.dram_tensor("J", [128, 128], F16, kind="ExternalInput")
    id_h = nc.dram_tensor("ident", [128, 128], F16, kind="ExternalInput")
    # wq|wk|wv concatenated on the last axis: [L, H, 3*FEAT]; norm1_w folded
    wqkv_h = nc.dram_tensor("wqkvT", [L, H, 3 * FEAT], F16, kind="ExternalInput")
    woT_h = nc.dram_tensor("woT", [L, FEAT, H], F16, kind="ExternalInput")
    # w1|w3 interleaved by m-group (norm2_w folded):
    # [w1 0:384 | w3 0:384 | w1 384:704 | w3 384:704]
    w13_h = nc.dram_tensor("w13T", [L, H, 2 * PC], F16, kind="ExternalInput")
    w2T_h = nc.dram_tensor("w2T", [L, PC, H], F16, kind="ExternalInput")
    owT_h = nc.dram_tensor("owT", [H, VC], F16, kind="ExternalInput")
    out_h = nc.dram_tensor("logits", [1, VC], F32, kind="ExternalOutput")

    MW = [128] * (KP - 1) + [PC - 128 * (KP - 1)]   # 128 x5, 64
    MG_OFF = [0, 384]
    MG_WID = [384, PC - 384]

    def coll_rs_ag(in_ap, mid_ap, out_ap):
        nc.gpsimd.collective_compute(
            "ReduceScatter", ALU.add, replica_groups=[list(range(NC))],
            ins=[in_ap.opt()], outs=[mid_ap.opt()])
        nc.gpsimd.collective_compute(
            "AllGather", ALU.bypass, replica_groups=[list(range(NC))],
            ins=[mid_ap.opt()], outs=[out_ap.opt()])

    def coll_ar(ins_ap, outs_ap):
        nc.gpsimd.collective_compute(
            "AllReduce", ALU.add, replica_groups=[list(range(NC))],
            ins=[ins_ap.opt()], outs=[outs_ap.opt()])

    from contextlib import ExitStack

    with tile.TileContext(nc) as tc, ExitStack() as _ctx:
        ec = _ctx.enter_context
        p_resid = ec(tc.tile_pool(name="resid", bufs=1))
        p_const = ec(tc.tile_pool(name="consts", bufs=1))
        p_row = ec(tc.tile_pool(name="row", bufs=1))
        p_inv = ec(tc.tile_pool(name="invp", bufs=4))
        p_big = ec(tc.tile_pool(name="big", bufs=1))
        p_vs = ec(tc.tile_pool(name="vsn", bufs=1))
        p_pt = ec(tc.tile_pool(name="ptile", bufs=3))
        p_f32 = ec(tc.tile_pool(name="f32t", bufs=3))
        p_t512 = ec(tc.tile_pool(name="t512", bufs=3))
        p_sq = ec(tc.tile_pool(name="sqp", bufs=3))
        p_stg = ec(tc.tile_pool(name="stage", bufs=4))
        p_w13 = ec(tc.tile_pool(name="w13", bufs=3))
        p_w2 = ec(tc.tile_pool(name="w2p", bufs=2))
        p_swig = ec(tc.tile_pool(name="swig", bufs=6))
        p_ar = ec(tc.tile_pool(name="ars", bufs=4))
        psum = ec(tc.tile_pool(name="psum", bufs=2, space="PSUM"))
        dram = ec(tc.tile_pool(name="dram", bufs=4, space="DRAM"))

        xT = p_resid.tile([128, KH * S], F16, tag="xT")
        for hc in range(KH):
            nc.sync.dma_start(xT[:, ts(hc, S)], xT_h.ap()[ts(hc, 128), :])

        C_s = p_const.tile([128, S], F16, tag="C")
        nc.sync.dma_start(C_s[:], C_h.ap())
        S_s = p_const.tile([128, S], F16, tag="S")
        nc.sync.dma_start(S_s[:], S_h.ap())
        J_r = p_const.tile([128, 128], F16, tag="J")
        nc.sync.dma_start(J_r[:], J_h.ap())
        id_r = p_const.tile([128, 128], F16, tag="id")
        nc.sync.dma_start(id_r[:], id_h.ap())
        mdiag_s = p_const.tile([128, 4 * 512], F16, tag="mdiag")
        nc.sync.dma_start(mdiag_s[:], mdiag_h.ap())
        mlast_s = p_const.tile([128, 2], F32, tag="mlast")
        nc.sync.dma_start(mlast_s[:], mlast_h.ap())
        ones_f = p_const.tile([128, 1], F32, tag="o1f")
        nc.vector.memset(ones_f[:], 1.0)
        ones_ch = p_const.tile([128, 1], F16, tag="o1h")
        nc.vector.tensor_copy(ones_ch[:], ones_f[:])
        ones_rf = p_const.tile([1, 128], F32, tag="orf")
        nc.vector.memset(ones_rf[:], 1.0)
        ones_rh = p_const.tile([1, 128], F16, tag="orh")
        nc.vector.tensor_copy(ones_rh[:], ones_rf[:])
        one_h = p_const.tile([1, 1], F16, tag="oneh")
        nc.vector.tensor_copy(one_h[:], ones_f[:1, :])
        ones_mh = p_const.tile([128, 128], F16, tag="omh")
        nc.vector.memset(ones_mh[:], 1.0)
        eps_t = p_const.tile([1, 1], F32, tag="eps")
        nc.vector.memset(eps_t[:], EPS)
        eps_p = p_const.tile([128, 1], F32, tag="epsp")
        nc.vector.memset(eps_p[:], EPS)

        # per-token-half 1/rms tiles, rebuilt by each qkv_half call
        inv1 = {}   # layer-norm1 inv for rope/v-scale: [1, 512] f16 per tk

        def norm_inv(tk):
            """1/rms over tokens [tk*512, +512) of the current xT."""
            ssum = psum.tile([1, 512], F32, tag="acc", bufs=6, name="ssum")
            for hc in range(KH):
                sq = p_sq.tile([128, 512], F16, tag="sq", name="sq")
                sl = slice(hc * S + tk * 512, hc * S + tk * 512 + 512)
                nc.vector.tensor_mul(sq[:], xT[:, sl], xT[:, sl])
                nc.tensor.matmul(ssum[:], ones_ch[:], sq[:],
                                 start=(hc == 0), stop=(hc == KH - 1))
            rms = p_inv.tile([1, 512], F32, tag="rmsn", name="rmsn")
            nc.scalar.activation(rms[:], ssum[:], AF.Sqrt,
                                 bias=eps_t[:], scale=1.0 / H)
            invf = p_inv.tile([1, 512], F32, tag="invf", name="invf")
            nc.vector.reciprocal_approx_fast(invf[:], rms[:])
            inv = p_inv.tile([1, 512], F16, tag="inv", name="ninv")
            nc.vector.tensor_copy(inv[:], invf[:])
            return inv

        def bcast(inv):
            """broadcast [1,512] -> [128,512] f16 via PE."""
            bc_ps = psum.tile([128, 512], F32, tag="ps512", name="bcps")
            nc.tensor.matmul(bc_ps[:], ones_rh[:], inv[:], start=True, stop=True)
            bc = p_f32.tile([128, 512], F16, tag="bc", name="bcs")
            nc.scalar.activation(bc[:], bc_ps[:], AF.Copy)
            return bc

        def qkv_half(l_, tk, q_s, k_s, vT_s):
            """Projections for token half tk of layer l_ from RAW xT (norm
            deferred). Writes [:, mt*S + tk*512]; stores inv1[tk]."""
            inv1[tk] = norm_inv(tk)
            last_ = (l_ == L - 1)
            qp = {}
            if not last_ or tk == 1:
                qp = {mt: psum.tile([128, 512], F32, tag="acc", bufs=6,
                                    name=f"qp{mt}") for mt in range(2)}
            kp = {mt: psum.tile([128, 512], F32, tag="acc", bufs=6,
                                name=f"kp{mt}") for mt in range(2)}
            vp = {mt: psum.tile([128, 512], F32, tag="acc", bufs=6,
                                name=f"vp{mt}") for mt in range(2)}
            for hc in range(KH):
                xsl = xT[:, hc * S + tk * 512: hc * S + tk * 512 + 512]
                wt = p_w13.tile([128, 3 * FEAT], F16, tag="w13", name="wt")
                nc.sync.dma_start(wt[:], wqkv_h.ap()[l_, ts(hc, 128), :])
                st, sp = (hc == 0), (hc == KH - 1)
                for mt in range(2):
                    if mt in qp:
                        if last_:
                            nc.tensor.matmul(
                                qp[mt][:, :2], wt[:, ts(mt, 128)],
                                xT[:, hc * S + S - 2: hc * S + S],
                                start=st, stop=sp)
                        else:
                            nc.tensor.matmul(qp[mt][:], wt[:, ts(mt, 128)],
                                             xsl, start=st, stop=sp)
                    nc.tensor.matmul(kp[mt][:], wt[:, 256 + mt * 128: 384 + mt * 128],
                                     xsl, start=st, stop=sp)
                    nc.tensor.matmul(vp[mt][:], wt[:, 512 + mt * 128: 640 + mt * 128],
                                     xsl, start=st, stop=sp)
            for mt in range(2):
                off = mt * S + tk * 512
                if mt in qp:
                    if last_:
                        nc.scalar.activation(q_s[:, mt * S + S - 2: mt * S + S],
                                             qp[mt][:, :2], AF.Copy)
                    else:
                        nc.scalar.activation(q_s[:, off:off + 512], qp[mt][:],
                                             AF.Copy)
                nc.scalar.activation(k_s[:, off:off + 512], kp[mt][:], AF.Copy)
                nc.vector.tensor_copy(vT_s[:, off:off + 512], vp[mt][:])

        # ---- layer 0 QKV prologue ----
        cur_q = p_big.tile([128, 2 * S], F16, tag="q0", name="q0")
        cur_k = p_big.tile([128, 2 * S], F16, tag="k0", name="k0")
        cur_vT = p_big.tile([128, 2 * S], F16, tag="vT0", name="vT0")
        attn_s = p_big.tile([128, 2 * S], F16, tag="attn", name="attn")
        for tk in range(2):
            qkv_half(0, tk, cur_q, cur_k, cur_vT)

        for l in range(L):
            last = (l == L - 1)
            q_s, k_s, vT_s = cur_q, cur_k, cur_vT

            # fold 1/rms into rope tables: Cb[n] = C*bc_n, Sb[n] = S*bc_n
            Cb, Sb = {}, {}
            invT = p_inv.tile([128, 8], F32, tag="invT", name="invT")
            for n in range(2):
                bc = bcast(inv1[n])
                Cb[n] = p_inv.tile([128, 512], F16, tag="Cb", name=f"Cb{n}")
                nc.vector.tensor_mul(Cb[n][:], C_s[:, ts(n, 512)], bc[:])
                Sb[n] = p_inv.tile([128, 512], F16, tag="Sb", name=f"Sb{n}")
                nc.vector.tensor_mul(Sb[n][:], S_s[:, ts(n, 512)], bc[:])
                for b in range(4):
                    it_ps = psum.tile([128, 1], F32, tag="ps512", name="itps")
                    nc.tensor.matmul(it_ps[:], inv1[n][:, ts(b, 128)],
                                     one_h[:], start=True, stop=True)
                    nc.scalar.activation(invT[:, n * 4 + b: n * 4 + b + 1],
                                         it_ps[:], AF.Copy)

            # RoPE in place:  out = Cb*x + Sb*(J@x)  (1/rms folded in Cb/Sb)
            def rope_slice(t_s, mt, n, c0, w):
                """rope tokens [n*512+c0, +w) of chunk mt of t_s."""
                sl = slice(mt * S + n * 512 + c0, mt * S + n * 512 + c0 + w)
                csl = slice(c0, c0 + w)
                j_ps = psum.tile([128, 512], F32, tag="ps512", name="jps")
                nc.tensor.matmul(j_ps[:, :w], J_r[:], t_s[:, sl],
                                 start=True, stop=True)
                tmp = p_t512.tile([128, 512], F16, tag="t512r", name="rtmp")
                nc.vector.tensor_mul(tmp[:, :w], Cb[n][:, csl], t_s[:, sl])
                nc.vector.tensor_mul(t_s[:, sl], j_ps[:, :w], Sb[n][:, csl])
                nc.vector.tensor_add(t_s[:, sl], t_s[:, sl], tmp[:, :w])

            for mt in range(2):
                for n in range(2):
                    rope_slice(k_s, mt, n, 0, 512)
                    if not last:
                        rope_slice(q_s, mt, n, 0, 512)
            if last:
                for mt in range(2):
                    rope_slice(q_s, mt, 1, 510, 2)

            # V -> natural layout [tok, feat] via PE transpose; 1/rms folded
            # into the evacuation (tensor_scalar per-partition mul).
            v_s = p_vs.tile([128, 8 * FEAT], F16, tag="v", name="vs")
            for mt in range(2):
                for tb in range(8):
                    tp = psum.tile([128, 128], F16, tag="ps512", name="tp")
                    nc.tensor.transpose(
                        tp[:], vT_s[:, mt * S + tb * 128: mt * S + tb * 128 + 128],
                        id_r[:])
                    nc.vector.tensor_scalar_mul(
                        v_s[:, tb * FEAT + mt * 128: tb * FEAT + mt * 128 + 128],
                        tp[:], invT[:, tb:tb + 1])

            if last:
                # only the last token's query matters (2-wide for ISA).
                for h in range(2):
                    at1 = psum.tile([128, 2], F32, tag="acc", bufs=6, name="at1")
                    rs1 = psum.tile([128, 2], F32, tag="acc", bufs=6, name="rs1")
                    for kc in range(8):
                        sc1 = psum.tile([128, 2], F32, tag="ps512", name="sc1")
                        nc.tensor.matmul(
                            sc1[:],
                            k_s[:, h * S + kc * 128: h * S + kc * 128 + 128],
                            q_s[:, h * S + S - 2: h * S + S],
                            start=True, stop=True)
                        pt1 = p_t512.tile([128, 2], F16, tag="mk1", name="pt1")
                        if kc == 7:
                            ex1 = p_t512.tile([128, 2], F32, tag="mk1",
                                              name="ex1")
                            nc.vector.scalar_tensor_tensor(
                                ex1[:], sc1[:], INV_SCALE, mlast_s[:],
                                op0=ALU.mult, op1=ALU.add)
                            nc.scalar.activation(pt1[:], ex1[:], AF.Exp)
                        else:
                            nc.scalar.activation(pt1[:], sc1[:], AF.Exp,
                                                 scale=INV_SCALE)
                        st, sp = (kc == 0), (kc == 7)
                        nc.tensor.matmul(
                            at1[:],
                            v_s[:, kc * FEAT + h * 128: kc * FEAT + h * 128 + 128],
                            pt1[:], start=st, stop=sp)
                        nc.tensor.matmul(rs1[:], ones_mh[:], pt1[:],
                                         start=st, stop=sp)
                    inva = p_t512.tile([128, 2], F32, tag="mk1", name="inva")
                    nc.vector.reciprocal_approx_fast(inva[:], rs1[:])
                    nc.vector.tensor_mul(
                        attn_s[:, h * S + S - 2: h * S + S], at1[:], inva[:])

                # wo -> [H,2] AllReduce -> residual add (last token)
                ar_in = dram.tile([128, 2 * KH], F16, tag="arinL", name="arinL")
                ar_out = dram.tile([128, 2 * KH], F16, tag="aroutL",
                                   addr_space="Shared", name="aroutL")
                arwL = p_ar.tile([128, 2 * KH], F16, tag="arL", name="arwL")
                woL = [p_stg.tile([128, H], F16, tag="stg",
                                  name=f"woL{i}") for i in range(2)]
                for fc in range(2):
                    nc.sync.dma_start(woL[fc][:], woT_h.ap()[l, ts(fc, 128), :])
                for hc in range(KH):
                    poL = psum.tile([128, 2], F32, tag="ps512", name="poL")
                    for fc in range(2):
                        nc.tensor.matmul(
                            poL[:], woL[fc][:, ts(hc, 128)],
                            attn_s[:, fc * S + S - 2: fc * S + S],
                            start=(fc == 0), stop=(fc == 1))
                    nc.scalar.activation(arwL[:, 2 * hc: 2 * hc + 2], poL[:],
                                         AF.Copy)
                nc.sync.dma_start(ar_in[:], arwL[:])
                coll_ar(ar_in[:], ar_out[:])
                arrL = p_ar.tile([128, 2 * KH], F16, tag="arL", name="arrL")
                nc.sync.dma_start(arrL[:], ar_out[:])
                for hc in range(KH):
                    nc.vector.tensor_add(
                        xT[:, hc * S + S - 2: hc * S + S],
                        xT[:, hc * S + S - 2: hc * S + S],
                        arrL[:, 2 * hc: 2 * hc + 2])

                # norm2 + FFN on the last 2 tokens (n2w folded into w13)
                sqL = p_row.tile([128, 2 * KH], F16, tag="sql2")
                for hc in range(KH):
                    col = hc * S + S - 2
                    nc.vector.tensor_mul(sqL[:, 2 * hc:2 * hc + 2],
                                         xT[:, col:col + 2], xT[:, col:col + 2])
                ssL = psum.tile([128, 2 * KH], F32, tag="ps512", name="ssL")
                nc.tensor.matmul(ssL[:], ones_mh[:], sqL[:],
                                 start=True, stop=True)
                ssr = p_row.tile([128, 2], F32, tag="ssr")
                nc.vector.reduce_sum(
                    ssr[:], ssL[:].rearrange("p (c two) -> p two c", two=2),
                    axis=mybir.AxisListType.X)
                rmsL = p_row.tile([128, 2], F32, tag="rmsL")
                nc.scalar.activation(rmsL[:], ssr[:], AF.Sqrt,
                                     bias=eps_p[:], scale=1.0 / H)
                invL = p_row.tile([128, 2], F32, tag="invLc")
                nc.vector.reciprocal_approx_fast(invL[:], rmsL[:])
                hnL = p_row.tile([128, 2 * KH], F16, tag="hnL")
                for hc in range(KH):
                    col = hc * S + S - 2
                    nc.vector.tensor_mul(hnL[:, 2 * hc:2 * hc + 2],
                                         xT[:, col:col + 2], invL[:])
                swigL = p_row.tile([128, 2 * KP], F16, tag="swL")
                for mg in range(2):
                    mts = [0, 1, 2] if mg == 0 else [3, 4, 5]
                    w_off, w_wid = MG_OFF[mg], MG_WID[mg]
                    gL = {mt: psum.tile([128, 2], F32, tag="acc", bufs=6,
                                        name=f"gL{mt}") for mt in mts}
                    uL = {mt: psum.tile([128, 2], F32, tag="acc", bufs=6,
                                        name=f"uL{mt}") for mt in mts}
                    for hc in range(KH):
                        wt13 = p_w13.tile([128, 2 * 384], F16, tag="w13",
                                          name="wt13L")
                        nc.sync.dma_start(
                            wt13[:, :2 * w_wid],
                            w13_h.ap()[l, ts(hc, 128),
                                       2 * w_off: 2 * w_off + 2 * w_wid])
                        st, sp = (hc == 0), (hc == KH - 1)
                        for i, mt in enumerate(mts):
                            w = min(128, w_wid - i * 128)
                            nc.tensor.matmul(
                                gL[mt][:w, :], wt13[:, i * 128: i * 128 + w],
                                hnL[:, 2 * hc:2 * hc + 2], start=st, stop=sp)
                            nc.tensor.matmul(
                                uL[mt][:w, :],
                                wt13[:, w_wid + i * 128: w_wid + i * 128 + w],
                                hnL[:, 2 * hc:2 * hc + 2], start=st, stop=sp)
                    for mt in mts:
                        kw = MW[mt]
                        gsL = p_row.tile([128, 2], F16, tag="gsL")
                        nc.scalar.activation(gsL[:kw, :], gL[mt][:kw, :], AF.Silu)
                        nc.vector.tensor_mul(swigL[:kw, 2 * mt:2 * mt + 2],
                                             uL[mt][:kw, :], gsL[:kw, :])
                ar2_in = dram.tile([128, 2 * KH], F16, tag="arinL",
                                   name="ar2inL")
                ar2_out = dram.tile([128, 2 * KH], F16, tag="aroutL",
                                    addr_space="Shared", name="ar2outL")
                arw2L = p_ar.tile([128, 2 * KH], F16, tag="arL", name="arw2L")
                for hcb in range(4):
                    p2L = [psum.tile([128, 2], F32, tag="acc", bufs=6,
                                     name=f"p2L{i}") for i in range(4)]
                    for kc in range(KP):
                        kw = MW[kc]
                        w2_t = p_w2.tile([128, 512], F16, tag="w2",
                                         name="w2tL")
                        nc.sync.dma_start(
                            w2_t[:kw, :],
                            w2T_h.ap()[l, kc * 128: kc * 128 + kw,
                                       hcb * 512: hcb * 512 + 512])
                        for hh in range(4):
                            nc.tensor.matmul(
                                p2L[hh][:], w2_t[:kw, ts(hh, 128)],
                                swigL[:kw, 2 * kc:2 * kc + 2],
                                start=(kc == 0), stop=(kc == KP - 1))
                    for hh in range(4):
                        hc = hcb * 4 + hh
                        nc.scalar.activation(arw2L[:, 2 * hc: 2 * hc + 2],
                                             p2L[hh][:], AF.Copy)
                nc.sync.dma_start(ar2_in[:], arw2L[:])
                coll_ar(ar2_in[:], ar2_out[:])
                arr2L = p_ar.tile([128, 2 * KH], F16, tag="arL", name="arr2L")
                nc.sync.dma_start(arr2L[:], ar2_out[:])
                for hc in range(KH):
                    nc.vector.tensor_add(
                        xT[:, hc * S + S - 2: hc * S + S],
                        xT[:, hc * S + S - 2: hc * S + S],
                        arr2L[:, 2 * hc: 2 * hc + 2])
                continue

            # ---- non-last layer: attention for both halves, then the
            # token-half-pipelined tail (wo->AR1->norm2->FFN->AR2->next QKV)
            for tk in range(2):
                # causal: query half tk only attends key blocks kc*128 <
                # (tk+1)*512. Fully-visible blocks exp straight from PSUM;
                # diagonal blocks add the preloaded [128,512] mask pattern d.
                if tk == 0:
                    blocks = [(kc, kc) for kc in range(4)]
                else:
                    blocks = ([(kc, None) for kc in range(4)]
                              + [(kc, kc - 4) for kc in range(4, 8)])
                nb = len(blocks)
                at_ps, rs_ps = {}, {}
                for h in range(2):
                    at_ps[h] = psum.tile([128, 512], F32, tag="acc", bufs=6,
                                         name=f"atp{h}")
                    rs_ps[h] = psum.tile([1, 512], F32, tag="acc", bufs=6,
                                         name=f"rsp{h}")
                for bi, (kc, d) in enumerate(blocks):
                    sc = {}
                    for h in range(2):
                        sc[h] = psum.tile([128, 512], F32, tag="ps512",
                                          name=f"scp{h}")
                        nc.tensor.matmul(
                            sc[h][:],
                            k_s[:, h * S + kc * 128: h * S + kc * 128 + 128],
                            q_s[:, h * S + tk * 512: h * S + tk * 512 + 512],
                            start=True, stop=True)
                    pts = {}
                    for h in range(2):
                        pt = p_pt.tile([128, 512], F16, tag="pt", name="ptl")
                        if d is None:
                            nc.scalar.activation(pt[:], sc[h][:], AF.Exp,
                                                 scale=INV_SCALE)
                        else:
                            ex = p_t512.tile([128, 512], F32, tag="t512f",
                                             name="ex")
                            nc.vector.scalar_tensor_tensor(
                                ex[:], sc[h][:], INV_SCALE,
                                mdiag_s[:, ts(d, 512)],
                                op0=ALU.mult, op1=ALU.add)
                            nc.scalar.activation(pt[:], ex[:], AF.Exp)
                        pts[h] = pt
                    st, sp = (bi == 0), (bi == nb - 1)
                    for h in range(2):
                        nc.tensor.matmul(
                            at_ps[h][:],
                            v_s[:, kc * FEAT + h * 128: kc * FEAT + h * 128 + 128],
                            pts[h][:], start=st, stop=sp)
                        nc.tensor.matmul(rs_ps[h][:], ones_ch[:], pts[h][:],
                                         start=st, stop=sp)
                for h in range(2):
                    inv = p_row.tile([1, 512], F32, tag="sinv", name="ainv")
                    nc.vector.reciprocal_approx_fast(inv[:], rs_ps[h][:])
                    invh = p_row.tile([1, 512], F16, tag="sinvh", name="ainvh")
                    nc.vector.tensor_copy(invh[:], inv[:])
                    ib_ps = psum.tile([128, 512], F32, tag="ps512", name="ibp")
                    nc.tensor.matmul(ib_ps[:], ones_rh[:], invh[:],
                                     start=True, stop=True)
                    ib_s = p_f32.tile([128, 512], F16, tag="bc", name="ibs")
                    nc.scalar.activation(ib_s[:], ib_ps[:], AF.Copy)
                    nc.vector.tensor_mul(
                        attn_s[:, h * S + tk * 512: h * S + tk * 512 + 512],
                        at_ps[h][:], ib_s[:])

            ar1_bufs = []
            ar2_bufs = []
            wo_t = None
            for tk in range(2):
                # wo projection for this token half; partials staged as one
                # contiguous [128, KH*512] f16 block
                ar_in = dram.tile([128, KH * 512], F16, tag="arin", name="arin")
                ar_out = dram.tile([128, KH * 512], F16, tag="arout",
                                   addr_space="Shared", name="arout")
                ar1_bufs.append((ar_in, ar_out))
                arw = [p_stg.tile([128, 8 * 512], F16, tag="stg",
                                  name=f"arw{i}") for i in range(2)]
                if wo_t is None:
                    wo_t = [p_stg.tile([128, H], F16, tag="stg",
                                       name=f"wof{i}") for i in range(2)]
                    for fc in range(2):
                        nc.sync.dma_start(wo_t[fc][:],
                                          woT_h.ap()[l, ts(fc, 128), :])
                for hc in range(KH):
                    po = psum.tile([128, 512], F32, tag="ps512", name="po")
                    for fc in range(2):
                        nc.tensor.matmul(
                            po[:], wo_t[fc][:, ts(hc, 128)],
                            attn_s[:, fc * S + tk * 512: fc * S + tk * 512 + 512],
                            start=(fc == 0), stop=(fc == 1))
                    nc.scalar.activation(arw[hc // 8][:, ts(hc % 8, 512)],
                                         po[:], AF.Copy)
                for i in range(2):
                    nc.sync.dma_start(ar_in[:, ts(i, 8 * 512)], arw[i][:])
                ar_mid = dram.tile([16, KH * 512], F16, tag="armid",
                                   name="armid")
                coll_rs_ag(ar_in[:], ar_mid[:], ar_out[:])

            for tk in range(2):
                ar_in, ar_out = ar1_bufs[tk]
                arr = [p_stg.tile([128, 8 * 512], F16, tag="stg",
                                  name=f"arr{i}") for i in range(2)]
                for i in range(2):
                    nc.sync.dma_start(arr[i][:], ar_out[:, ts(i, 8 * 512)])
                for hc in range(KH):
                    nc.vector.tensor_add(
                        xT[:, hc * S + tk * 512: hc * S + tk * 512 + 512],
                        xT[:, hc * S + tk * 512: hc * S + tk * 512 + 512],
                        arr[hc // 8][:, ts(hc % 8, 512)])

                # norm2 + FFN for this half (n2w folded into w13)
                inv2 = norm_inv(tk)
                bc2 = bcast(inv2)
                swig = [p_swig.tile([128, 512], F16, tag="sw",
                                    name=f"swig{i}") for i in range(KP)]
                for mg in range(2):
                    mts = [0, 1, 2] if mg == 0 else [3, 4, 5]
                    w_off, w_wid = MG_OFF[mg], MG_WID[mg]
                    gp = {mt: psum.tile([128, 512], F32, tag="acc", bufs=6,
                                        name=f"gp{mt}") for mt in mts}
                    up = {mt: psum.tile([128, 512], F32, tag="acc", bufs=6,
                                        name=f"up{mt}") for mt in mts}
                    for hc in range(KH):
                        hn = p_sq.tile([128, 512], F16, tag="hn", name="hn")
                        nc.vector.tensor_mul(
                            hn[:],
                            xT[:, hc * S + tk * 512: hc * S + tk * 512 + 512],
                            bc2[:])
                        wt13 = p_w13.tile([128, 2 * 384], F16, tag="w13",
                                          name="wt13")
                        nc.sync.dma_start(
                            wt13[:, :2 * w_wid],
                            w13_h.ap()[l, ts(hc, 128),
                                       2 * w_off: 2 * w_off + 2 * w_wid])
                        st, sp = (hc == 0), (hc == KH - 1)
                        for i, mt in enumerate(mts):
                            w = min(128, w_wid - i * 128)
                            nc.tensor.matmul(
                                gp[mt][:w, :], wt13[:, i * 128: i * 128 + w],
                                hn[:], start=st, stop=sp)
                            nc.tensor.matmul(
                                up[mt][:w, :],
                                wt13[:, w_wid + i * 128: w_wid + i * 128 + w],
                                hn[:], start=st, stop=sp)
                    for i, mt in enumerate(mts):
                        w = MW[mt]
                        gs = p_t512.tile([128, 512], F16, tag="t512f", name="gs")
                        nc.scalar.activation(gs[:w, :], gp[mt][:w, :], AF.Silu)
                        nc.vector.tensor_mul(
                            swig[mt][:w, :], up[mt][:w, :], gs[:w, :])

                # down projection for this half
                ar2_in = dram.tile([128, KH * 512], F16, tag="arin",
                                   name="ar2in")
                ar2_out = dram.tile([128, KH * 512], F16, tag="arout",
                                    addr_space="Shared", name="ar2out")
                ar2_bufs.append((ar2_in, ar2_out))
                arw2 = [p_stg.tile([128, 8 * 512], F16, tag="stg",
                                   name=f"arw2{i}") for i in range(2)]
                for hcb in range(4):
                    p2 = [psum.tile([128, 512], F32, tag="acc", bufs=6,
                                    name=f"p2p{i}") for i in range(4)]
                    for kc in range(KP):
                        kw = MW[kc]
                        w2_t = p_w2.tile([128, 512], F16, tag="w2", name="w2t")
                        nc.sync.dma_start(
                            w2_t[:kw, :],
                            w2T_h.ap()[l, kc * 128: kc * 128 + kw,
                                       hcb * 512: hcb * 512 + 512])
                        for hh in range(4):
                            nc.tensor.matmul(
                                p2[hh][:], w2_t[:kw, ts(hh, 128)],
                                swig[kc][:kw, :],
                                start=(kc == 0), stop=(kc == KP - 1))
                    for hh in range(4):
                        hc = hcb * 4 + hh
                        nc.scalar.activation(arw2[hc // 8][:, ts(hc % 8, 512)],
                                             p2[hh][:], AF.Copy)
                for i in range(2):
                    nc.sync.dma_start(ar2_in[:, ts(i, 8 * 512)], arw2[i][:])
                ar2_mid = dram.tile([16, KH * 512], F16, tag="armid",
                                    name="ar2mid")
                coll_rs_ag(ar2_in[:], ar2_mid[:], ar2_out[:])

            for tk in range(2):
                ar2_in, ar2_out = ar2_bufs[tk]
                arr2 = [p_stg.tile([128, 8 * 512], F16, tag="stg",
                                   name=f"arr2{i}") for i in range(2)]
                for i in range(2):
                    nc.sync.dma_start(arr2[i][:], ar2_out[:, ts(i, 8 * 512)])
                for hc in range(KH):
                    nc.vector.tensor_add(
                        xT[:, hc * S + tk * 512: hc * S + tk * 512 + 512],
                        xT[:, hc * S + tk * 512: hc * S + tk * 512 + 512],
                        arr2[hc // 8][:, ts(hc % 8, 512)])

                # next layer's QKV for this half (overlaps the other AR)
                if l + 1 < L:
                    qkv_half(l + 1, tk, cur_q, cur_k, cur_vT)

        # ======== final norm (last token only) + logits ========
        # final_norm_w is folded into out_w; 1/rms applied as a scalar at the
        # end (single token).
        sq_l = p_row.tile([128, KH], F16, tag="sql")
        for hc in range(KH):
            col = hc * S + S - 1
            nc.vector.tensor_mul(sq_l[:, hc:hc + 1], xT[:, col:col + 1],
                                 xT[:, col:col + 1])
        sl_ps = psum.tile([1, KH], F32, tag="ps512", name="slps")
        nc.tensor.matmul(sl_ps[:], ones_ch[:], sq_l[:], start=True, stop=True)
        ssc = p_row.tile([1, 1], F32, tag="ssc")
        nc.vector.reduce_sum(ssc[:], sl_ps[:], axis=mybir.AxisListType.X)
        rms_l = p_row.tile([1, 1], F32, tag="rmsl")
        nc.scalar.activation(rms_l[:], ssc[:], AF.Sqrt, bias=eps_t[:],
                             scale=1.0 / H)
        inv_l = p_row.tile([1, 1], F32, tag="invl")
        nc.vector.reciprocal_approx_fast(inv_l[:], rms_l[:])
        xnl = p_row.tile([128, KH], F16, tag="xnl")
        for hc in range(KH):
            col = hc * S + S - 1
            nc.vector.tensor_copy(xnl[:, hc:hc + 1], xT[:, col:col + 1])
        # vocab in 2 passes of 4x500 columns; out_w streamed in [128, 2000]
        # tiles (512KB DMAs) with 4 live [1,500] psum accumulators per pass.
        for vp in range(2):
            voff = vp * 2000
            lg_ps = [psum.tile([1, 500], F32, tag="acc", bufs=6, name=f"lgps{n}")
                     for n in range(4)]
            for hc in range(KH):
                ow_t = p_stg.tile([128, 2000], F16, tag="owt", bufs=6,
                                  name="owt")
                nc.sync.dma_start(
                    ow_t[:], owT_h.ap()[ts(hc, 128), voff: voff + 2000])
                for n in range(4):
                    nc.tensor.matmul(lg_ps[n][:], xnl[:, hc: hc + 1],
                                     ow_t[:, ts(n, 500)],
                                     start=(hc == 0), stop=(hc == KH - 1))
            for n in range(4):
                lg = p_row.tile([1, 500], F32, tag="lg")
                nc.scalar.activation(lg[:], lg_ps[n][:], AF.Copy,
                                     scale=inv_l[:])
                nc.sync.dma_start(
                    out_h.ap()[:, voff + n * 500: voff + n * 500 + 500], lg[:])

    nc.compile()
    return nc


def _shard(inputs):
    x = np.asarray(inputs["x"], np.float32)
    mask = np.asarray(inputs["attn_mask"], np.float32)
    cos = np.asarray(inputs["cos"], np.float32).reshape(S, HD // 2)
    sin = np.asarray(inputs["sin"], np.float32).reshape(S, HD // 2)
    n1 = np.asarray(inputs["norm1_w"], np.float32)[:L]
    n2 = np.asarray(inputs["norm2_w"], np.float32)[:L]
    fw = np.asarray(inputs["final_norm_w"], np.float32)
    wq = np.asarray(inputs["wq"], np.float32)[:L]
    wk = np.asarray(inputs["wk"], np.float32)[:L]
    wv = np.asarray(inputs["wv"], np.float32)[:L]
    wo = np.asarray(inputs["wo"], np.float32)[:L]
    w1 = np.asarray(inputs["w1"], np.float32)[:L]
    w3 = np.asarray(inputs["w3"], np.float32)[:L]
    w2 = np.asarray(inputs["w2"], np.float32)[:L]
    ow = np.asarray(inputs["out_w"], np.float32)

    # fold the norm weights into the following projections (exact):
    #   rmsnorm(x, w) @ W.T == (x * inv_rms) @ (W * w).T
    wq = wq * n1[:, None, :]
    wk = wk * n1[:, None, :]
    wv = wv * n1[:, None, :]
    w1 = w1 * n2[:, None, :]
    w3 = w3 * n2[:, None, :]
    ow = ow * fw[None, :]

    xT = np.ascontiguousarray(x[0].T).astype(np.float16)
    mlast = np.ascontiguousarray(mask[0].T[7 * 128:8 * 128, S - 2: S])
    # diagonal-block causal masks: pattern d covers key block kc with
    # kc % 4 == d against a 512-query half; 0 where visible else NEG
    kl = np.arange(128)[:, None]
    qq = np.arange(512)[None, :]
    mdiag = np.concatenate(
        [np.where(kl + 128 * d <= qq, 0.0, NEG) for d in range(4)],
        axis=1).astype(np.float16)
    C = np.empty((128, S), np.float32)
    C[0::2] = cos.T
    C[1::2] = cos.T
    Sm = np.empty((128, S), np.float32)
    Sm[0::2] = -sin.T
    Sm[1::2] = sin.T
    J = np.zeros((128, 128), np.float16)
    idx = np.arange(0, 128, 2)
    J[idx, idx + 1] = 1.0
    J[idx + 1, idx] = 1.0
    ident = np.eye(128, dtype=np.float16)

    common = dict(xT=xT, mlast=mlast, mdiag=mdiag,
                  Cr=C.astype(np.float16), Sr=Sm.astype(np.float16),
                  J=J, ident=ident)
    in_maps = []
    for c in range(NC):
        fs = slice(c * FEAT, (c + 1) * FEAT)
        ps = slice(c * PC, (c + 1) * PC)
        vs = slice(c * VC, (c + 1) * VC)
        m = dict(common)
        wqT = wq[:, fs, :].transpose(0, 2, 1)
        wkT = wk[:, fs, :].transpose(0, 2, 1)
        wvT = wv[:, fs, :].transpose(0, 2, 1)
        m["wqkvT"] = np.ascontiguousarray(
            np.concatenate([wqT, wkT, wvT], axis=2)).astype(np.float16)
        m["woT"] = np.ascontiguousarray(
            wo[:, :, fs].transpose(0, 2, 1)).astype(np.float16)
        w1T = w1[:, ps, :].transpose(0, 2, 1)
        w3T = w3[:, ps, :].transpose(0, 2, 1)
        m["w13T"] = np.ascontiguousarray(np.concatenate(
            [w1T[:, :, 0:384], w3T[:, :, 0:384],
             w1T[:, :, 384:], w3T[:, :, 384:]], axis=2)).astype(np.float16)
        m["w2T"] = np.ascontiguousarray(
            w2[:, :, ps].transpose(0, 2, 1)).astype(np.float16)
        m["owT"] = np.ascontiguousarray(ow[vs, :].T).astype(np.float16)
        in_maps.append(m)
    return in_maps


def kernel(**inputs) -> np.ndarray:
    from concourse import bass_utils

    if "nc" not in _STATE:
        _STATE["nc"] = _build()
    in_maps = _shard(inputs)
    res = bass_utils.run_bass_kernel_spmd(
        _STATE["nc"], in_maps, core_ids=list(range(NC)))
    out = np.concatenate(
        [res.results[c]["logits"] for c in range(NC)], axis=1)
    return out.astype(np.float32)


# revision 21
# speedup vs baseline: 1.2600x; 1.1844x over previous
"""Trainium2 Bass kernel: 4-layer decoder prefill (S=1024, H=2048, NH=16, HD=128,
FFN=5632, V=32000), tensor-parallel over 8 NeuronCores.

- Megatron TP over 8 cores: wq/wk/wv/w1/w3 sharded on output dim (2 heads /
  704 ffn rows per core), wo/w2 sharded on input dim (partials -> RS+AG),
  out_w sharded over vocab (4000 rows/core); only the last token's logits are
  computed.
- The residual stream lives TRANSPOSED in SBUF (xT: [H on partition-chunks,
  S free]); weights are pre-transposed on the host so every matmul contracts
  over the partition dim with no on-device weight transposes.
- RMS normalization is DEFERRED past the QKV projection: the norm weights are
  folded into wq/wk/wv/w1/w3/out_w on the host, and the per-token 1/rms is
  folded into the RoPE cos/sin tables (rope is linear) and into the V
  PE-transpose evacuation (tensor_scalar mul), so the QKV matmuls read the
  raw residual xT directly with no elementwise prologue.
- 1/rms comes from a single ACT Rsqrt; softmax 1/sum stays on DVE f32r.
- All activations are f16 (2x DVE modes); matmuls accumulate in f32 PSUM.
"""

import os
import sys

sys.path.insert(0, "/opt/trn_rl_repo")

import numpy as np

L = int(os.environ.get("KERNEL_DEV_L", "4"))
B, S, H, NH, HD = 1, 1024, 2048, 16, 128
V, P = 32000, 5632
NC = 8
FEAT = H // NC          # 256 q/k/v features per core (2 heads)
PC = P // NC            # 704 ffn rows per core
VC = V // NC            # 4000 vocab rows per core
KH = H // 128           # 16 H-chunks
KP = (PC + 127) // 128  # 6 pc-chunks (last is 64)
EPS = 1e-5
SCALE = float(np.sqrt(HD))
INV_SCALE = 1.0 / SCALE
NEG = -30000.0          # f16-safe mask value

_STATE = {}


def _build():
    import concourse.bass as bass
    import concourse.bacc as bacc
    from concourse import tile, mybir

    F32 = mybir.dt.float32
    F32R = mybir.dt.float32r
    F16 = mybir.dt.float16
    AF = mybir.ActivationFunctionType
    ALU = mybir.AluOpType
    ts = bass.ts

    nc = bacc.Bacc("TRN2", target_bir_lowering=False, debug=False, num_devices=NC)

    xT_h = nc.dram_tensor("xT", [H, S], F16, kind="ExternalInput")
    mlast_h = nc.dram_tensor("mlast", [128, 2], F32, kind="ExternalInput")
    mdiag_h = nc.dram_tensor("mdiag", [128, 4 * 512], F16, kind="ExternalInput")
    C_h = nc.dram_tensor("Cr", [128, S], F16, kind="ExternalInput")
    S_h = nc.dram_tensor("Sr", [128, S], F16, kind="ExternalInput")
    J_h = nc.dram_tensor("J", [128, 128], F16, kind="ExternalInput")
    id_h = nc.dram_tensor("ident", [128, 128], F16, kind="ExternalInput")
    # wq|wk|wv concatenated on the last axis: [L, H, 3*FEAT]; norm1_w folded
    wqkv_h = nc.dram_tensor("wqkvT", [L, H, 3 * FEAT], F16, kind="ExternalInput")
    woT_h = nc.dram_tensor("woT", [L, FEAT, H], F16, kind="ExternalInput")
    # w1|w3 interleaved by m-group (norm2_w folded):
    # [w1 0:384 | w3 0:384 | w1 384:704 | w3 384:704]
    w13_h = nc.dram_tensor("w13T", [L, H, 2 * PC], F16, kind="ExternalInput")
    w2T_h = nc.dram_tensor("w2T", [L, PC, H], F16, kind="ExternalInput")
    owT_h = nc.dram_tensor("owT", [H, VC], F16, kind="ExternalInput")
    out_h = nc.dram_tensor("logits", [1, VC], F32, kind="ExternalOutput")

    MW = [128] * (KP - 1) + [PC - 128 * (KP - 1)]   # 128 x5, 64
    MG_OFF = [0, 384]
    MG_WID = [384, PC - 384]

    def coll_ar(ins_ap, outs_ap):
        nc.gpsimd.collective_compute(
            "AllReduce", ALU.add, replica_groups=[list(range(NC))],
            ins=[ins_ap.opt()], outs=[outs_ap.opt()])

    from contextlib import ExitStack

    with tile.TileContext(nc) as tc, ExitStack() as _ctx:
        ec = _ctx.enter_context
        p_resid = ec(tc.tile_pool(name="resid", bufs=1))
        p_const = ec(tc.tile_pool(name="consts", bufs=1))
        p_row = ec(tc.tile_pool(name="row", bufs=1))
        p_inv = ec(tc.tile_pool(name="invp", bufs=4))
        p_big = ec(tc.tile_pool(name="big", bufs=1))
        p_vs = ec(tc.tile_pool(name="vsn", bufs=1))
        p_pt = ec(tc.tile_pool(name="ptile", bufs=3))
        p_f32 = ec(tc.tile_pool(name="f32t", bufs=3))
        p_t512 = ec(tc.tile_pool(name="t512", bufs=3))
        p_sq = ec(tc.tile_pool(name="sqp", bufs=3))
        p_stg = ec(tc.tile_pool(name="stage", bufs=4))
        p_w13 = ec(tc.tile_pool(name="w13", bufs=4))
        p_w2 = ec(tc.tile_pool(name="w2p", bufs=3))
        p_swig = ec(tc.tile_pool(name="swig", bufs=6))
        p_ar = ec(tc.tile_pool(name="ars", bufs=4))
        psum = ec(tc.tile_pool(name="psum", bufs=2, space="PSUM"))
        dram = ec(tc.tile_pool(name="dram", bufs=4, space="DRAM"))

        xT = p_resid.tile([128, KH * S], F16, tag="xT")
        for hc in range(KH):
            nc.sync.dma_start(xT[:, ts(hc, S)], xT_h.ap()[ts(hc, 128), :])

        C_s = p_const.tile([128, S], F16, tag="C")
        nc.sync.dma_start(C_s[:], C_h.ap())
        S_s = p_const.tile([128, S], F16, tag="S")
        nc.sync.dma_start(S_s[:], S_h.ap())
        J_r = p_const.tile([128, 128], F16, tag="J")
        nc.sync.dma_start(J_r[:], J_h.ap())
        id_r = p_const.tile([128, 128], F16, tag="id")
        nc.sync.dma_start(id_r[:], id_h.ap())
        mdiag_s = p_const.tile([128, 4 * 512], F16, tag="mdiag")
        nc.sync.dma_start(mdiag_s[:], mdiag_h.ap())
        mlast_s = p_const.tile([128, 2], F32, tag="mlast")
        nc.sync.dma_start(mlast_s[:], mlast_h.ap())
        ones_f = p_const.tile([128, 1], F32, tag="o1f")
        nc.vector.memset(ones_f[:], 1.0)
        ones_ch = p_const.tile([128, 1], F16, tag="o1h")
        nc.vector.tensor_copy(ones_ch[:], ones_f[:])
        ones_rf = p_const.tile([1, 128], F32, tag="orf")
        nc.vector.memset(ones_rf[:], 1.0)
        ones_rh = p_const.tile([1, 128], F16, tag="orh")
        nc.vector.tensor_copy(ones_rh[:], ones_rf[:])
        one_h = p_const.tile([1, 1], F16, tag="oneh")
        nc.vector.tensor_copy(one_h[:], ones_f[:1, :])
        ones_mh = p_const.tile([128, 128], F16, tag="omh")
        nc.vector.memset(ones_mh[:], 1.0)
        eps_t = p_const.tile([1, 1], F32, tag="eps")
        nc.vector.memset(eps_t[:], EPS)
        eps_p = p_const.tile([128, 1], F32, tag="epsp")
        nc.vector.memset(eps_p[:], EPS)

        # per-token-half 1/rms tiles, rebuilt by each qkv_half call
        inv1 = {}   # layer-norm1 inv for rope/v-scale: [1, 512] f16 per tk

        def norm_inv(tk):
            """1/rms over tokens [tk*512, +512) of the current xT."""
            ssum = psum.tile([1, 512], F32, tag="acc", bufs=6, name="ssum")
            for hc in range(KH):
                sq = p_sq.tile([128, 512], F16, tag="sq", name="sq")
                sl = slice(hc * S + tk * 512, hc * S + tk * 512 + 512)
                nc.vector.tensor_mul(sq[:], xT[:, sl], xT[:, sl])
                nc.tensor.matmul(ssum[:], ones_ch[:], sq[:],
                                 start=(hc == 0), stop=(hc == KH - 1))
            rms = p_inv.tile([1, 512], F32, tag="rmsn", name="rmsn")
            nc.scalar.activation(rms[:], ssum[:], AF.Sqrt,
                                 bias=eps_t[:], scale=1.0 / H)
            invf = p_inv.tile([1, 512], F32, tag="invf", name="invf")
            nc.vector.reciprocal_approx_fast(invf[:], rms[:])
            inv = p_inv.tile([1, 512], F16, tag="inv", name="ninv")
            nc.vector.tensor_copy(inv[:], invf[:])
            return inv

        def bcast(inv):
            """broadcast [1,512] -> [128,512] f16 via PE."""
            bc_ps = psum.tile([128, 512], F32, tag="ps512", name="bcps")
            nc.tensor.matmul(bc_ps[:], ones_rh[:], inv[:], start=True, stop=True)
            bc = p_f32.tile([128, 512], F16, tag="bc", name="bcs")
            nc.scalar.activation(bc[:], bc_ps[:], AF.Copy)
            return bc

        def qkv_half(l_, tk, q_s, k_s, vT_s):
            """Projections for token half tk of layer l_ from RAW xT (norm
            deferred). Writes [:, mt*S + tk*512]; stores inv1[tk]."""
            inv1[tk] = norm_inv(tk)
            last_ = (l_ == L - 1)
            qp = {}
            if not last_ or tk == 1:
                qp = {mt: psum.tile([128, 512], F32, tag="acc", bufs=6,
                                    name=f"qp{mt}") for mt in range(2)}
            kp = {mt: psum.tile([128, 512], F32, tag="acc", bufs=6,
                                name=f"kp{mt}") for mt in range(2)}
            vp = {mt: psum.tile([128, 512], F32, tag="acc", bufs=6,
                                name=f"vp{mt}") for mt in range(2)}
            for hc in range(KH):
                xsl = xT[:, hc * S + tk * 512: hc * S + tk * 512 + 512]
                wt = p_w13.tile([128, 3 * FEAT], F16, tag="w13", name="wt")
                nc.sync.dma_start(wt[:], wqkv_h.ap()[l_, ts(hc, 128), :])
                st, sp = (hc == 0), (hc == KH - 1)
                for mt in range(2):
                    if mt in qp:
                        if last_:
                            nc.tensor.matmul(
                                qp[mt][:, :2], wt[:, ts(mt, 128)],
                                xT[:, hc * S + S - 2: hc * S + S],
                                start=st, stop=sp)
                        else:
                            nc.tensor.matmul(qp[mt][:], wt[:, ts(mt, 128)],
                                             xsl, start=st, stop=sp)
                    nc.tensor.matmul(kp[mt][:], wt[:, 256 + mt * 128: 384 + mt * 128],
                                     xsl, start=st, stop=sp)
                    nc.tensor.matmul(vp[mt][:], wt[:, 512 + mt * 128: 640 + mt * 128],
                                     xsl, start=st, stop=sp)
            for mt in range(2):
                off = mt * S + tk * 512
                if mt in qp:
                    if last_:
                        nc.scalar.activation(q_s[:, mt * S + S - 2: mt * S + S],
                                             qp[mt][:, :2], AF.Copy)
                    else:
                        nc.scalar.activation(q_s[:, off:off + 512], qp[mt][:],
                                             AF.Copy)
                nc.scalar.activation(k_s[:, off:off + 512], kp[mt][:], AF.Copy)
                nc.vector.tensor_copy(vT_s[:, off:off + 512], vp[mt][:])

        # ---- layer 0 QKV prologue ----
        cur_q = p_big.tile([128, 2 * S], F16, tag="q0", name="q0")
        cur_k = p_big.tile([128, 2 * S], F16, tag="k0", name="k0")
        cur_vT = p_big.tile([128, 2 * S], F16, tag="vT0", name="vT0")
        attn_s = p_big.tile([128, 2 * S], F16, tag="attn", name="attn")
        for tk in range(2):
            qkv_half(0, tk, cur_q, cur_k, cur_vT)

        for l in range(L):
            last = (l == L - 1)
            q_s, k_s, vT_s = cur_q, cur_k, cur_vT

            # fold 1/rms into rope tables: Cb[n] = C*bc_n, Sb[n] = S*bc_n
            Cb, Sb = {}, {}
            invT = p_inv.tile([128, 8], F32, tag="invT", name="invT")
            for n in range(2):
                bc = bcast(inv1[n])
                Cb[n] = p_inv.tile([128, 512], F16, tag="Cb", name=f"Cb{n}")
                nc.vector.tensor_mul(Cb[n][:], C_s[:, ts(n, 512)], bc[:])
                Sb[n] = p_inv.tile([128, 512], F16, tag="Sb", name=f"Sb{n}")
                nc.vector.tensor_mul(Sb[n][:], S_s[:, ts(n, 512)], bc[:])
                for b in range(4):
                    it_ps = psum.tile([128, 1], F32, tag="ps512", name="itps")
                    nc.tensor.matmul(it_ps[:], inv1[n][:, ts(b, 128)],
                                     one_h[:], start=True, stop=True)
                    nc.scalar.activation(invT[:, n * 4 + b: n * 4 + b + 1],
                                         it_ps[:], AF.Copy)

            # RoPE in place:  out = Cb*x + Sb*(J@x)  (1/rms folded in Cb/Sb)
            def rope_slice(t_s, mt, n, c0, w):
                """rope tokens [n*512+c0, +w) of chunk mt of t_s."""
                sl = slice(mt * S + n * 512 + c0, mt * S + n * 512 + c0 + w)
                csl = slice(c0, c0 + w)
                j_ps = psum.tile([128, 512], F32, tag="ps512", name="jps")
                nc.tensor.matmul(j_ps[:, :w], J_r[:], t_s[:, sl],
                                 start=True, stop=True)
                tmp = p_t512.tile([128, 512], F16, tag="t512r", name="rtmp")
                nc.vector.tensor_mul(tmp[:, :w], Cb[n][:, csl], t_s[:, sl])
                nc.vector.tensor_mul(t_s[:, sl], j_ps[:, :w], Sb[n][:, csl])
                nc.vector.tensor_add(t_s[:, sl], t_s[:, sl], tmp[:, :w])

            for mt in range(2):
                for n in range(2):
                    rope_slice(k_s, mt, n, 0, 512)
                    if not last:
                        rope_slice(q_s, mt, n, 0, 512)
            if last:
                for mt in range(2):
                    rope_slice(q_s, mt, 1, 510, 2)

            # V -> natural layout [tok, feat] via PE transpose; 1/rms folded
            # into the evacuation (tensor_scalar per-partition mul).
            v_s = p_vs.tile([128, 8 * FEAT], F16, tag="v", name="vs")
            for mt in range(2):
                for tb in range(8):
                    tp = psum.tile([128, 128], F16, tag="ps512", name="tp")
                    nc.tensor.transpose(
                        tp[:], vT_s[:, mt * S + tb * 128: mt * S + tb * 128 + 128],
                        id_r[:])
                    nc.vector.tensor_scalar_mul(
                        v_s[:, tb * FEAT + mt * 128: tb * FEAT + mt * 128 + 128],
                        tp[:], invT[:, tb:tb + 1])

            if last:
                # only the last token's query matters (2-wide for ISA).
                for h in range(2):
                    at1 = psum.tile([128, 2], F32, tag="acc", bufs=6, name="at1")
                    rs1 = psum.tile([128, 2], F32, tag="acc", bufs=6, name="rs1")
                    for kc in range(8):
                        sc1 = psum.tile([128, 2], F32, tag="ps512", name="sc1")
                        nc.tensor.matmul(
                            sc1[:],
                            k_s[:, h * S + kc * 128: h * S + kc * 128 + 128],
                            q_s[:, h * S + S - 2: h * S + S],
                            start=True, stop=True)
                        pt1 = p_t512.tile([128, 2], F16, tag="mk1", name="pt1")
                        if kc == 7:
                            ex1 = p_t512.tile([128, 2], F32, tag="mk1",
                                              name="ex1")
                            nc.vector.scalar_tensor_tensor(
                                ex1[:], sc1[:], INV_SCALE, mlast_s[:],
                                op0=ALU.mult, op1=ALU.add)
                            nc.scalar.activation(pt1[:], ex1[:], AF.Exp)
                        else:
                            nc.scalar.activation(pt1[:], sc1[:], AF.Exp,
                                                 scale=INV_SCALE)
                        st, sp = (kc == 0), (kc == 7)
                        nc.tensor.matmul(
                            at1[:],
                            v_s[:, kc * FEAT + h * 128: kc * FEAT + h * 128 + 128],
                            pt1[:], start=st, stop=sp)
                        nc.tensor.matmul(rs1[:], ones_mh[:], pt1[:],
                                         start=st, stop=sp)
                    inva = p_t512.tile([128, 2], F32, tag="mk1", name="inva")
                    nc.vector.reciprocal_approx_fast(inva[:], rs1[:])
                    nc.vector.tensor_mul(
                        attn_s[:, h * S + S - 2: h * S + S], at1[:], inva[:])

                # wo -> [H,2] AllReduce -> residual add (last token)
                ar_in = dram.tile([128, 2 * KH], F16, tag="arinL", name="arinL")
                ar_out = dram.tile([128, 2 * KH], F16, tag="aroutL",
                                   addr_space="Shared", name="aroutL")
                arwL = p_ar.tile([128, 2 * KH], F16, tag="arL", name="arwL")
                woL = [p_stg.tile([128, H], F16, tag="stg",
                                  name=f"woL{i}") for i in range(2)]
                for fc in range(2):
                    nc.sync.dma_start(woL[fc][:], woT_h.ap()[l, ts(fc, 128), :])
                for hc in range(KH):
                    poL = psum.tile([128, 2], F32, tag="ps512", name="poL")
                    for fc in range(2):
                        nc.tensor.matmul(
                            poL[:], woL[fc][:, ts(hc, 128)],
                            attn_s[:, fc * S + S - 2: fc * S + S],
                            start=(fc == 0), stop=(fc == 1))
                    nc.scalar.activation(arwL[:, 2 * hc: 2 * hc + 2], poL[:],
                                         AF.Copy)
                nc.sync.dma_start(ar_in[:], arwL[:])
                coll_ar(ar_in[:], ar_out[:])
                arrL = p_ar.tile([128, 2 * KH], F16, tag="arL", name="arrL")
                nc.sync.dma_start(arrL[:], ar_out[:])
                for hc in range(KH):
                    nc.vector.tensor_add(
                        xT[:, hc * S + S - 2: hc * S + S],
                        xT[:, hc * S + S - 2: hc * S + S],
                        arrL[:, 2 * hc: 2 * hc + 2])

                # norm2 + FFN on the last 2 tokens (n2w folded into w13)
                sqL = p_row.tile([128, 2 * KH], F16, tag="sql2")
                for hc in range(KH):
                    col = hc * S + S - 2
                    nc.vector.tensor_mul(sqL[:, 2 * hc:2 * hc + 2],
                                         xT[:, col:col + 2], xT[:, col:col + 2])
                ssL = psum.tile([128, 2 * KH], F32, tag="ps512", name="ssL")
                nc.tensor.matmul(ssL[:], ones_mh[:], sqL[:],
                                 start=True, stop=True)
                ssr = p_row.tile([128, 2], F32, tag="ssr")
                nc.vector.reduce_sum(
                    ssr[:], ssL[:].rearrange("p (c two) -> p two c", two=2),
                    axis=mybir.AxisListType.X)
                rmsL = p_row.tile([128, 2], F32, tag="rmsL")
                nc.scalar.activation(rmsL[:], ssr[:], AF.Sqrt,
                                     bias=eps_p[:], scale=1.0 / H)
                invL = p_row.tile([128, 2], F32, tag="invLc")
                nc.vector.reciprocal_approx_fast(invL[:], rmsL[:])
                hnL = p_row.tile([128, 2 * KH], F16, tag="hnL")
                for hc in range(KH):
                    col = hc * S + S - 2
                    nc.vector.tensor_mul(hnL[:, 2 * hc:2 * hc + 2],
                                         xT[:, col:col + 2], invL[:])
                swigL = p_row.tile([128, 2 * KP], F16, tag="swL")
                for mg in range(2):
                    mts = [0, 1, 2] if mg == 0 else [3, 4, 5]
                    w_off, w_wid = MG_OFF[mg], MG_WID[mg]
                    gL = {mt: psum.tile([128, 2], F32, tag="acc", bufs=6,
                                        name=f"gL{mt}") for mt in mts}
                    uL = {mt: psum.tile([128, 2], F32, tag="acc", bufs=6,
                                        name=f"uL{mt}") for mt in mts}
                    for hc in range(KH):
                        wt13 = p_w13.tile([128, 2 * 384], F16, tag="w13",
                                          name="wt13L")
                        nc.sync.dma_start(
                            wt13[:, :2 * w_wid],
                            w13_h.ap()[l, ts(hc, 128),
                                       2 * w_off: 2 * w_off + 2 * w_wid])
                        st, sp = (hc == 0), (hc == KH - 1)
                        for i, mt in enumerate(mts):
                            w = min(128, w_wid - i * 128)
                            nc.tensor.matmul(
                                gL[mt][:w, :], wt13[:, i * 128: i * 128 + w],
                                hnL[:, 2 * hc:2 * hc + 2], start=st, stop=sp)
                            nc.tensor.matmul(
                                uL[mt][:w, :],
                                wt13[:, w_wid + i * 128: w_wid + i * 128 + w],
                                hnL[:, 2 * hc:2 * hc + 2], start=st, stop=sp)
                    for mt in mts:
                        kw = MW[mt]
                        gsL = p_row.tile([128, 2], F16, tag="gsL")
                        nc.scalar.activation(gsL[:kw, :], gL[mt][:kw, :], AF.Silu)
                        nc.vector.tensor_mul(swigL[:kw, 2 * mt:2 * mt + 2],
                                             uL[mt][:kw, :], gsL[:kw, :])
                ar2_in = dram.tile([128, 2 * KH], F16, tag="arinL",
                                   name="ar2inL")
                ar2_out = dram.tile([128, 2 * KH], F16, tag="aroutL",
                                    addr_space="Shared", name="ar2outL")
                arw2L = p_ar.tile([128, 2 * KH], F16, tag="arL", name="arw2L")
                for hcb in range(4):
                    p2L = [psum.tile([128, 2], F32, tag="acc", bufs=6,
                                     name=f"p2L{i}") for i in range(4)]
                    for kc in range(KP):
                        kw = MW[kc]
                        w2_t = p_w2.tile([128, 512], F16, tag="w2",
                                         name="w2tL")
                        nc.sync.dma_start(
                            w2_t[:kw, :],
                            w2T_h.ap()[l, kc * 128: kc * 128 + kw,
                                       hcb * 512: hcb * 512 + 512])
                        for hh in range(4):
                            nc.tensor.matmul(
                                p2L[hh][:], w2_t[:kw, ts(hh, 128)],
                                swigL[:kw, 2 * kc:2 * kc + 2],
                                start=(kc == 0), stop=(kc == KP - 1))
                    for hh in range(4):
                        hc = hcb * 4 + hh
                        nc.scalar.activation(arw2L[:, 2 * hc: 2 * hc + 2],
                                             p2L[hh][:], AF.Copy)
                nc.sync.dma_start(ar2_in[:], arw2L[:])
                coll_ar(ar2_in[:], ar2_out[:])
                arr2L = p_ar.tile([128, 2 * KH], F16, tag="arL", name="arr2L")
                nc.sync.dma_start(arr2L[:], ar2_out[:])
                for hc in range(KH):
                    nc.vector.tensor_add(
                        xT[:, hc * S + S - 2: hc * S + S],
                        xT[:, hc * S + S - 2: hc * S + S],
                        arr2L[:, 2 * hc: 2 * hc + 2])
                continue

            # ---- non-last layer: token-half-pipelined
            # (attn->wo->AR1 | norm2->FFN->AR2->next QKV); attention of half
            # tk1 runs on the PE while AR1(tk0) is on the collective stream.
            def attention(tk):
                # causal: query half tk only attends key blocks kc*128 <
                # (tk+1)*512. Fully-visible blocks exp straight from PSUM;
                # diagonal blocks add the preloaded [128,512] mask pattern d.
                if tk == 0:
                    blocks = [(kc, kc) for kc in range(4)]
                else:
                    blocks = ([(kc, None) for kc in range(4)]
                              + [(kc, kc - 4) for kc in range(4, 8)])
                nb = len(blocks)
                at_ps, rs_ps = {}, {}
                for h in range(2):
                    at_ps[h] = psum.tile([128, 512], F32, tag="acc", bufs=6,
                                         name=f"atp{h}")
                    rs_ps[h] = psum.tile([1, 512], F32, tag="acc", bufs=6,
                                         name=f"rsp{h}")
                for bi, (kc, d) in enumerate(blocks):
                    sc = {}
                    for h in range(2):
                        sc[h] = psum.tile([128, 512], F32, tag="ps512",
                                          name=f"scp{h}")
                        nc.tensor.matmul(
                            sc[h][:],
                            k_s[:, h * S + kc * 128: h * S + kc * 128 + 128],
                            q_s[:, h * S + tk * 512: h * S + tk * 512 + 512],
                            start=True, stop=True)
                    pts = {}
                    for h in range(2):
                        pt = p_pt.tile([128, 512], F16, tag="pt", name="ptl")
                        if d is None:
                            nc.scalar.activation(pt[:], sc[h][:], AF.Exp,
                                                 scale=INV_SCALE)
                        else:
                            ex = p_t512.tile([128, 512], F32, tag="t512f",
                                             name="ex")
                            nc.vector.scalar_tensor_tensor(
                                ex[:], sc[h][:], INV_SCALE,
                                mdiag_s[:, ts(d, 512)],
                                op0=ALU.mult, op1=ALU.add)
                            nc.scalar.activation(pt[:], ex[:], AF.Exp)
                        pts[h] = pt
                    st, sp = (bi == 0), (bi == nb - 1)
                    for h in range(2):
                        nc.tensor.matmul(
                            at_ps[h][:],
                            v_s[:, kc * FEAT + h * 128: kc * FEAT + h * 128 + 128],
                            pts[h][:], start=st, stop=sp)
                        nc.tensor.matmul(rs_ps[h][:], ones_ch[:], pts[h][:],
                                         start=st, stop=sp)
                for h in range(2):
                    inv = p_row.tile([1, 512], F32, tag="sinv", name="ainv")
                    nc.vector.reciprocal_approx_fast(inv[:], rs_ps[h][:])
                    invh = p_row.tile([1, 512], F16, tag="sinvh", name="ainvh")
                    nc.vector.tensor_copy(invh[:], inv[:])
                    ib_ps = psum.tile([128, 512], F32, tag="ps512", name="ibp")
                    nc.tensor.matmul(ib_ps[:], ones_rh[:], invh[:],
                                     start=True, stop=True)
                    ib_s = p_f32.tile([128, 512], F16, tag="bc", name="ibs")
                    nc.scalar.activation(ib_s[:], ib_ps[:], AF.Copy)
                    nc.vector.tensor_mul(
                        attn_s[:, h * S + tk * 512: h * S + tk * 512 + 512],
                        at_ps[h][:], ib_s[:])

            ar1_bufs = []
            ar2_bufs = []
            wo_t = None
            for tk in range(2):
                attention(tk)
                # wo projection for this token half; partials staged as one
                # contiguous [128, KH*512] f16 block
                ar_in = dram.tile([128, KH * 512], F16, tag="arin", name="arin")
                ar_out = dram.tile([128, KH * 512], F16, tag="arout",
                                   addr_space="Shared", name="arout")
                ar1_bufs.append((ar_in, ar_out))
                arw = [p_stg.tile([128, 8 * 512], F16, tag="stg",
                                  name=f"arw{i}") for i in range(2)]
                if wo_t is None:
                    wo_t = [p_stg.tile([128, H], F16, tag="stg",
                                       name=f"wof{i}") for i in range(2)]
                    for fc in range(2):
                        nc.sync.dma_start(wo_t[fc][:],
                                          woT_h.ap()[l, ts(fc, 128), :])
                for hc in range(KH):
                    po = psum.tile([128, 512], F32, tag="ps512", name="po")
                    for fc in range(2):
                        nc.tensor.matmul(
                            po[:], wo_t[fc][:, ts(hc, 128)],
                            attn_s[:, fc * S + tk * 512: fc * S + tk * 512 + 512],
                            start=(fc == 0), stop=(fc == 1))
                    nc.scalar.activation(arw[hc // 8][:, ts(hc % 8, 512)],
                                         po[:], AF.Copy)
                for i in range(2):
                    nc.sync.dma_start(ar_in[:, ts(i, 8 * 512)], arw[i][:])
                coll_ar(ar_in[:], ar_out[:])

            for tk in range(2):
                ar_in, ar_out = ar1_bufs[tk]
                arr = [p_stg.tile([128, 8 * 512], F16, tag="stg",
                                  name=f"arr{i}") for i in range(2)]
                for i in range(2):
                    nc.sync.dma_start(arr[i][:], ar_out[:, ts(i, 8 * 512)])
                for hc in range(KH):
                    nc.vector.tensor_add(
                        xT[:, hc * S + tk * 512: hc * S + tk * 512 + 512],
                        xT[:, hc * S + tk * 512: hc * S + tk * 512 + 512],
                        arr[hc // 8][:, ts(hc % 8, 512)])

                # norm2 + FFN for this half (n2w folded into w13)
                inv2 = norm_inv(tk)
                bc2 = bcast(inv2)
                swig = [p_swig.tile([128, 512], F16, tag="sw",
                                    name=f"swig{i}") for i in range(KP)]
                for mg in range(2):
                    mts = [0, 1, 2] if mg == 0 else [3, 4, 5]
                    w_off, w_wid = MG_OFF[mg], MG_WID[mg]
                    gp = {mt: psum.tile([128, 512], F32, tag="acc", bufs=6,
                                        name=f"gp{mt}") for mt in mts}
                    up = {mt: psum.tile([128, 512], F32, tag="acc", bufs=6,
                                        name=f"up{mt}") for mt in mts}
                    for hc in range(KH):
                        hn = p_sq.tile([128, 512], F16, tag="hn", name="hn")
                        nc.vector.tensor_mul(
                            hn[:],
                            xT[:, hc * S + tk * 512: hc * S + tk * 512 + 512],
                            bc2[:])
                        wt13 = p_w13.tile([128, 2 * 384], F16, tag="w13",
                                          name="wt13")
                        nc.sync.dma_start(
                            wt13[:, :2 * w_wid],
                            w13_h.ap()[l, ts(hc, 128),
                                       2 * w_off: 2 * w_off + 2 * w_wid])
                        st, sp = (hc == 0), (hc == KH - 1)
                        for i, mt in enumerate(mts):
                            w = min(128, w_wid - i * 128)
                            nc.tensor.matmul(
                                gp[mt][:w, :], wt13[:, i * 128: i * 128 + w],
                                hn[:], start=st, stop=sp)
                            nc.tensor.matmul(
                                up[mt][:w, :],
                                wt13[:, w_wid + i * 128: w_wid + i * 128 + w],
                                hn[:], start=st, stop=sp)
                    for i, mt in enumerate(mts):
                        w = MW[mt]
                        gs = p_t512.tile([128, 512], F16, tag="t512f", name="gs")
                        nc.scalar.activation(gs[:w, :], gp[mt][:w, :], AF.Silu)
                        nc.vector.tensor_mul(
                            swig[mt][:w, :], up[mt][:w, :], gs[:w, :])

                # down projection for this half
                ar2_in = dram.tile([128, KH * 512], F16, tag="arin",
                                   name="ar2in")
                ar2_out = dram.tile([128, KH * 512], F16, tag="arout",
                                    addr_space="Shared", name="ar2out")
                ar2_bufs.append((ar2_in, ar2_out))
                arw2 = [p_stg.tile([128, 8 * 512], F16, tag="stg",
                                   name=f"arw2{i}") for i in range(2)]
                for hcb in range(4):
                    p2 = [psum.tile([128, 512], F32, tag="acc", bufs=6,
                                    name=f"p2p{i}") for i in range(4)]
                    for kc in range(KP):
                        kw = MW[kc]
                        w2_t = p_w2.tile([128, 512], F16, tag="w2", name="w2t")
                        nc.sync.dma_start(
                            w2_t[:kw, :],
                            w2T_h.ap()[l, kc * 128: kc * 128 + kw,
                                       hcb * 512: hcb * 512 + 512])
                        for hh in range(4):
                            nc.tensor.matmul(
                                p2[hh][:], w2_t[:kw, ts(hh, 128)],
                                swig[kc][:kw, :],
                                start=(kc == 0), stop=(kc == KP - 1))
                    for hh in range(4):
                        hc = hcb * 4 + hh
                        nc.scalar.activation(arw2[hc // 8][:, ts(hc % 8, 512)],
                                             p2[hh][:], AF.Copy)
                for i in range(2):
                    nc.sync.dma_start(ar2_in[:, ts(i, 8 * 512)], arw2[i][:])
                coll_ar(ar2_in[:], ar2_out[:])

            for tk in range(2):
                ar2_in, ar2_out = ar2_bufs[tk]
                arr2 = [p_stg.tile([128, 8 * 512], F16, tag="stg",
                                   name=f"arr2{i}") for i in range(2)]
                for i in range(2):
                    nc.sync.dma_start(arr2[i][:], ar2_out[:, ts(i, 8 * 512)])
                for hc in range(KH):
                    nc.vector.tensor_add(
                        xT[:, hc * S + tk * 512: hc * S + tk * 512 + 512],
                        xT[:, hc * S + tk * 512: hc * S + tk * 512 + 512],
                        arr2[hc // 8][:, ts(hc % 8, 512)])

                # next layer's QKV for this half (overlaps the other AR)
                if l + 1 < L:
                    qkv_half(l + 1, tk, cur_q, cur_k, cur_vT)

        # ======== final norm (last token only) + logits ========
        # final_norm_w is folded into out_w; 1/rms applied as a scalar at the
        # end (single token).
        sq_l = p_row.tile([128, KH], F16, tag="sql")
        for hc in range(KH):
            col = hc * S + S - 1
            nc.vector.tensor_mul(sq_l[:, hc:hc + 1], xT[:, col:col + 1],
                                 xT[:, col:col + 1])
        sl_ps = psum.tile([1, KH], F32, tag="ps512", name="slps")
        nc.tensor.matmul(sl_ps[:], ones_ch[:], sq_l[:], start=True, stop=True)
        ssc = p_row.tile([1, 1], F32, tag="ssc")
        nc.vector.reduce_sum(ssc[:], sl_ps[:], axis=mybir.AxisListType.X)
        rms_l = p_row.tile([1, 1], F32, tag="rmsl")
        nc.scalar.activation(rms_l[:], ssc[:], AF.Sqrt, bias=eps_t[:],
                             scale=1.0 / H)
        inv_l = p_row.tile([1, 1], F32, tag="invl")
        nc.vector.reciprocal_approx_fast(inv_l[:], rms_l[:])
        xnl = p_row.tile([128, KH], F16, tag="xnl")
        for hc in range(KH):
            col = hc * S + S - 1
            nc.vector.tensor_copy(xnl[:, hc:hc + 1], xT[:, col:col + 1])
        # vocab in 2 passes of 4x500 columns; out_w streamed in [128, 2000]
        # tiles (512KB DMAs) with 4 live [1,500] psum accumulators per pass.
        for vp in range(2):
            voff = vp * 2000
            lg_ps = [psum.tile([1, 500], F32, tag="acc", bufs=6, name=f"lgps{n}")
                     for n in range(4)]
            for hc in range(KH):
                ow_t = p_stg.tile([128, 2000], F16, tag="owt", bufs=8,
                                  name="owt")
                nc.sync.dma_start(
                    ow_t[:], owT_h.ap()[ts(hc, 128), voff: voff + 2000])
                for n in range(4):
                    nc.tensor.matmul(lg_ps[n][:], xnl[:, hc: hc + 1],
                                     ow_t[:, ts(n, 500)],
                                     start=(hc == 0), stop=(hc == KH - 1))
            for n in range(4):
                lg = p_row.tile([1, 500], F32, tag="lg")
                nc.scalar.activation(lg[:], lg_ps[n][:], AF.Copy,
                                     scale=inv_l[:])
                nc.sync.dma_start(
                    out_h.ap()[:, voff + n * 500: voff + n * 500 + 500], lg[:])

    nc.compile()
    return nc


def _shard(inputs):
    x = np.asarray(inputs["x"], np.float32)
    mask = np.asarray(inputs["attn_mask"], np.float32)
    cos = np.asarray(inputs["cos"], np.float32).reshape(S, HD // 2)
    sin = np.asarray(inputs["sin"], np.float32).reshape(S, HD // 2)
    n1 = np.asarray(inputs["norm1_w"], np.float32)[:L]
    n2 = np.asarray(inputs["norm2_w"], np.float32)[:L]
    fw = np.asarray(inputs["final_norm_w"], np.float32)
    wq = np.asarray(inputs["wq"], np.float32)[:L]
    wk = np.asarray(inputs["wk"], np.float32)[:L]
    wv = np.asarray(inputs["wv"], np.float32)[:L]
    wo = np.asarray(inputs["wo"], np.float32)[:L]
    w1 = np.asarray(inputs["w1"], np.float32)[:L]
    w3 = np.asarray(inputs["w3"], np.float32)[:L]
    w2 = np.asarray(inputs["w2"], np.float32)[:L]
    ow = np.asarray(inputs["out_w"], np.float32)

    # fold the norm weights into the following projections (exact):
    #   rmsnorm(x, w) @ W.T == (x * inv_rms) @ (W * w).T
    wq = wq * n1[:, None, :]
    wk = wk * n1[:, None, :]
    wv = wv * n1[:, None, :]
    w1 = w1 * n2[:, None, :]
    w3 = w3 * n2[:, None, :]
    ow = ow * fw[None, :]

    xT = np.ascontiguousarray(x[0].T).astype(np.float16)
    mlast = np.ascontiguousarray(mask[0].T[7 * 128:8 * 128, S - 2: S])
    # diagonal-block causal masks: pattern d covers key block kc with
    # kc % 4 == d against a 512-query half; 0 where visible else NEG
    kl = np.arange(128)[:, None]
    qq = np.arange(512)[None, :]
    mdiag = np.concatenate(
        [np.where(kl + 128 * d <= qq, 0.0, NEG) for d in range(4)],
        axis=1).astype(np.float16)
    C = np.empty((128, S), np.float32)
    C[0::2] = cos.T
    C[1::2] = cos.T
    Sm = np.empty((128, S), np.float32)
    Sm[0::2] = -sin.T
    Sm[1::2] = sin.T
    J = np.zeros((128, 128), np.float16)
    idx = np.arange(0, 128, 2)
    J[idx, idx + 1] = 1.0
    J[idx + 1, idx] = 1.0
    ident = np.eye(128, dtype=np.float16)

    common = dict(xT=xT, mlast=mlast, mdiag=mdiag,
                  Cr=C.astype(np.float16), Sr=Sm.astype(np.float16),
                  J=J, ident=ident)
    in_maps = []
    for c in range(NC):
        fs = slice(c * FEAT, (c + 1) * FEAT)
        ps = slice(c * PC, (c + 1) * PC)
        vs = slice(c * VC, (c + 1) * VC)
        m = dict(common)
        wqT = wq[:, fs, :].transpose(0, 2, 1)
        wkT = wk[:, fs, :].transpose(0, 2, 1)
        wvT = wv[:, fs, :].transpose(0, 2, 1)
        m["wqkvT"] = np.ascontiguousarray(
            np.concatenate([wqT, wkT, wvT], axis=2)).astype(np.float16)
        m["woT"] = np.ascontiguousarray(
            wo[:, :, fs].transpose(0, 2, 1)).astype(np.float16)
        w1T = w1[:, ps, :].transpose(0, 2, 1)
        w3T = w3[:, ps, :].transpose(0, 2, 1)
        m["w13T"] = np.ascontiguousarray(np.concatenate(
            [w1T[:, :, 0:384], w3T[:, :, 0:384],
             w1T[:, :, 384:], w3T[:, :, 384:]], axis=2)).astype(np.float16)
        m["w2T"] = np.ascontiguousarray(
            w2[:, :, ps].transpose(0, 2, 1)).astype(np.float16)
        m["owT"] = np.ascontiguousarray(ow[vs, :].T).astype(np.float16)
        in_maps.append(m)
    return in_maps


def kernel(**inputs) -> np.ndarray:
    from concourse import bass_utils

    if "nc" not in _STATE:
        _STATE["nc"] = _build()
    in_maps = _shard(inputs)
    res = bass_utils.run_bass_kernel_spmd(
        _STATE["nc"], in_maps, core_ids=list(range(NC)))
    out = np.concatenate(
        [res.results[c]["logits"] for c in range(NC)], axis=1)
    return out.astype(np.float32)
